# revision 33
# baseline (speedup 1.0000x reference)
"""Trainium2 Bass kernel for nn_Decoder_43336220016932.

Luong-attention LSTM decoder with teacher forcing:
  out[b,t,:] = log_softmax(tanh([ctx_t, h_t] @ W_fc + b_fc))

v2 strategy (8 NeuronCores):
  - Vocab-sharded tensor parallel: core i owns W_fc[:, i*4000:(i+1)*4000]
    resident in SBUF as fp8 (x256), k-pair INTERLEAVED so the DoubleRow
    moving reads are contiguous. The serial LSTM recurrence is replicated
    on all cores; attention / FC / finalize are pipelined into it.
  - The recurrence h@W_h runs in fp8 DoubleRow (W_h x1024 fp8 stationary,
    fp8 h-state H8 = 16*h as moving): 32 LDW+MM pairs per step instead of
    64 bf16 pairs. PSUM holds z*16384; x@W_x (+b_lstm) is precomputed in
    blocks (bf16, x16384) into SBUF and added into PSUM in-place by DVE.
  - All sigmoids as tanh (sigmoid(x)=(1+tanh(x/2))/2) with affine factors
    folded (c kept as 2c, h as 2h); one ACT table set, no reloads.
  - State transposed ([U -> partitions, B -> free]); H8 (fp8 16h) feeds
    both the recurrence matmul and the FC; H (bf16 2h) feeds attention.
  - logits are tanh-bounded: sumexp uses exp(x-1); log_softmax needs one
    AllReduce(add) per batch of 8 chunks and no max pass.
  - Finalize (logits - logZ) reloads logits from DRAM, spread across
    Vector/Scalar, writes bf16 output via the gpsimd DMA queue.
"""
from collections import defaultdict
from contextlib import ExitStack

import numpy as np
import ml_dtypes

import concourse.bass as bass
import concourse.tile as tile
from concourse import bacc, mybir
from concourse.bass_utils import run_bass_kernel_spmd
from concourse.masks import make_identity

B, S, L, U, E, V = 32, 64, 64, 512, 256, 32000
T = S - 1                  # 63 decode steps
NC = 8                     # cores
VS = V // NC               # 4000 vocab shard per core
TB = T * B                 # 2016 (t, b) rows, t-major
G4U = 4 * U                # 2048
BL = B * L                 # 2048
SW = 256.0                 # fp8 scale on W_fc
SA = 16.0                  # fp8 scale on ctx / h
SP = SW * SA               # product scale on logits in PSUM
SWH = 1024.0               # fp8 scale on W_h
SZ = SWH * SA              # scale of z in PSUM
SZX = 2048.0               # fp8 scale on staged zx
SX = 128.0                 # fp8 scale on gathered x
SWX = 1024.0               # fp8 scale on W_x
AF = mybir.ActivationFunctionType
ALU = mybir.AluOpType
AX = mybir.AxisListType
PM = mybir.MatmulPerfMode
F32 = mybir.dt.float32
BF16 = mybir.dt.bfloat16
FP8 = mybir.dt.float8e4
I32 = mybir.dt.int32
HALF = VS // 2             # 2000
QSL = [(0, 512), (512, 512), (1024, 512), (1536, 464)]

_CACHE = {}


def build(n_cores=NC, fc_bias=False, lstm_bias=False):
    """Build the SPMD Bass program (same program on every core)."""
    nc = bacc.Bacc("TRN2", target_bir_lowering=False, debug=False,
                   num_devices=n_cores)

    # ---- external I/O ----
    tidx = nc.dram_tensor("tidx", [TB, 1], I32, kind="ExternalInput").ap()
    emb_bf = nc.dram_tensor("emb_bf", [V, E], BF16, kind="ExternalInput").ap()
    # W_x bf16 [128, (mi=16, j=2, m=128)] (x SWX)
    wx_q = nc.dram_tensor("wx_q", [128, 16 * 2 * 128], BF16,
                          kind="ExternalInput").ap()
    wh_q = nc.dram_tensor("wh_q", [128, 16 * 2 * 2 * 128], FP8,
                          kind="ExternalInput").ap()
    enc_bf = nc.dram_tensor("enc_bf", [128, 16 * U], BF16,
                            kind="ExternalInput").ap()
    enct_bf = nc.dram_tensor("enct_bf", [128, 4 * BL], BF16,
                             kind="ExternalInput").ap()
    wa_bf = nc.dram_tensor("wa_bf", [128, 4 * U], BF16,
                           kind="ExternalInput").ap()
    h0 = nc.dram_tensor("h0", [B, U], F32, kind="ExternalInput").ap()
    c0 = nc.dram_tensor("c0", [B, U], F32, kind="ExternalInput").ap()
    # W_fc fp8, [128, (j=4, v=VS, kk=2)] interleaved k-pairs
    wfc_q = nc.dram_tensor("wfc_q", [128, 4 * VS * 2], FP8,
                           kind="ExternalInput").ap()
    if lstm_bias:
        wxb_in = nc.dram_tensor("wxb_bf", [1, G4U], BF16,
                                kind="ExternalInput").ap()
    if fc_bias:
        bfc_in = nc.dram_tensor("bfc_bf", [1, VS], BF16,
                                kind="ExternalInput").ap()
    out = nc.dram_tensor("out", [B, T, VS], BF16, kind="ExternalOutput").ap()

    with tile.TileContext(nc) as tc, ExitStack() as perm:
        # ---------------- permanent pools ----------------
        konst = perm.enter_context(tc.tile_pool(name="konst", bufs=1))
        wpool = perm.enter_context(tc.tile_pool(name="wpool", bufs=1))
        hpool = perm.enter_context(tc.tile_pool(name="hpool", bufs=1))
        dram = perm.enter_context(tc.tile_pool(name="dram", bufs=1, space="DRAM"))
        stats = perm.enter_context(tc.tile_pool(name="stats", bufs=1))

        idt = konst.tile([128, 128], BF16)
        make_identity(nc, idt[:])
        negone = konst.tile([128, 1], F32)
        nc.vector.memset(negone[:], -1.0)
        idtf = konst.tile([128, 128], F32)
        make_identity(nc, idtf[:])
        ones_bf = konst.tile([1, 512], BF16)
        nc.vector.memset(ones_bf[:], 1.0)
        # fp8 identity x (SZ/SZX): folds the zx rescale into a PE-only add
        idt8 = konst.tile([128, 128], FP8)
        nc.vector.tensor_scalar_mul(idt8[:], idtf[:], SZ / SZX)

        # W_fc fp8 [p, (j, v, kk)]: per j-group the k-pair is interleaved
        wfc_sb = wpool.tile([128, 4 * VS * 2], FP8)
        wfc_k = wfc_sb[:].rearrange("p (j v k) -> p j k v", j=4, k=2)
        # W_h fp8 [p, (mi, kk, j, m)]
        wh_sb = wpool.tile([128, 16 * 2 * 2 * 128], FP8)
        wh_k = wh_sb[:].rearrange("p (mi kk j m) -> p mi kk j m",
                                  mi=16, kk=2, j=2)
        if fc_bias:
            bfc_row = wpool.tile([1, VS], BF16)

        # H: (2h).T history (bf16, attention scores); H8: fp8 x16 h
        # (recurrence + FC). col = k*2048 + slot*32 + b
        H = hpool.tile([128, 4 * 64 * B], BF16)
        Hk = H[:].rearrange("p (k s b) -> p k s b", k=4, s=64)
        H8 = hpool.tile([128, 4 * 64 * B], FP8)
        H8k = H8[:].rearrange("p (k s b) -> p k s b", k=4, s=64)
        H8r = H8[:].rearrange("p (k c) -> p k c", k=4)
        # G_ctx: ctx.T fp8 x16, col = k*2016 + t*32 + b
        Gc = hpool.tile([128, 4 * TB], FP8)
        Gck = Gc[:].rearrange("p (k t b) -> p k t b", k=4, t=T)
        Gcr = Gc[:].rearrange("p (k r) -> p k r", k=4)
        cT = hpool.tile([128, 128], F32)     # (2c).T state, col = k*32+b

        # Zx staged in SBUF: [p, (t, mi, b)] fp8, values x SZX
        zxt_sb = hpool.tile([128, T * 512], FP8)
        zxt_t = zxt_sb[:].rearrange("p (t c) -> p t c", t=T)
        zxt_blk = zxt_sb[:].rearrange("p (t m b) -> p m t b", t=T, m=16)

        # per-row ((t,b) grouped [128 x 16]) log-softmax stats.
        lsum_sb = stats.tile([128, 16], F32)   # local sum exp(x - 1)
        sg_sb = stats.tile([128, 16], F32)     # global sum
        logz_sb = stats.tile([128, 16], F32)   # ln(global sum)
        nlz_sb = stats.tile([128, 16], F32)    # -(1 + ln(global sum))
        nc.vector.memset(lsum_sb[:], 1.0)

        # DRAM scratch
        logits_d = dram.tile([16, 128, VS], BF16)    # tanh'd logits
        ccs_in = [dram.tile([128, 8], F32, name=f"cci{i}") for i in range(2)]
        ccs_out = [dram.tile([128, 8], F32, name=f"cco{i}") for i in range(2)]

        mwp = perm.enter_context(tc.tile_pool(name="midw", bufs=1))
        epT_sb = mwp.tile([128, 4 * BL], BF16)      # (enc @ Wa/2).T
        enc_sb = mwp.tile([128, 16 * U], BF16)      # 2 b per 64-row group

        d1w = perm.enter_context(tc.tile_pool(name="d1w", bufs=2))
        d1s = perm.enter_context(tc.tile_pool(name="d1s", bufs=2))
        sst = perm.enter_context(tc.tile_pool(name="sst", bufs=4))

        lg_tiles = {}
        ac_tiles = {}

        # =========== schedulable work units (emitted into R) ===========
        pools = {}

        def fc_half(mi, half):
            """FC chunk mi, vocab half (2 quarter-psum tiles):
            fp8 DoubleRow + tanh + exp."""
            r0 = mi * 128
            rows = min(128, TB - r0)
            if half == 0:
                lg_tiles[mi] = pools["lgp"].tile([128, VS], BF16, tag="lg",
                                                 name="lg")
            lg = lg_tiles[mi]
            acqs = []
            for q in range(2):
                base = half * HALF + q * 1000
                fcp = pools["fps"].tile([128, 1000], F32, tag="fc")
                for j in range(4):
                    lhs = Gcr[:, 2 * j: 2 * j + 2, r0:r0 + rows] if j < 2 \
                        else H8r[:, 2 * (j - 2): 2 * (j - 2) + 2,
                                 B + r0: B + r0 + rows]
                    for off, w in [(0, 512), (512, 488)]:
                        nc.tensor.matmul(
                            fcp[:rows, off:off + w], lhs,
                            wfc_k[:, j, :, base + off: base + off + w],
                            start=(j == 0), stop=(j == 3) and not fc_bias,
                            perf_mode=PM.DoubleRow)
                if fc_bias:
                    for off, w in [(0, 512), (512, 488)]:
                        nc.tensor.matmul(
                            fcp[:rows, off:off + w],
                            ones_bf[:1, :rows],
                            bfc_row[:1, base + off: base + off + w],
                            start=False, stop=True,
                            skip_group_check=True)
                qs = slice(base, base + 1000)
                nc.scalar.activation(lg[:rows, qs], fcp[:rows, :], AF.Tanh,
                                     scale=1.0 / SP)
                sc_ = pools["scr"].tile([128, 1000], BF16, tag="sc")
                acx = sst.tile([128, 1], F32, tag="ac")
                nc.scalar.activation(sc_[:rows, :], lg[:rows, qs], AF.Exp,
                                     bias=negone[:rows, :],
                                     accum_out=acx[:rows, :])
                nc.sync.dma_start(logits_d[mi, :rows, qs], lg[:rows, qs])
                acqs.append(acx)
            hsum = sst.tile([128, 1], F32, tag="hs")
            nc.vector.tensor_add(hsum[:rows, :], acqs[0][:rows, :],
                                 acqs[1][:rows, :])
            if half == 0:
                ac_tiles[mi] = hsum
            else:
                nc.vector.tensor_add(lsum_sb[:rows, mi:mi + 1],
                                     ac_tiles[mi][:rows, :], hsum[:rows, :])

        def ar_batch(bi):
            """AllReduce batch bi's sumexp; nlz = -(1 + ln S)."""
            ca, cb = 8 * bi, 8 * bi + 8
            nc.sync.dma_start(ccs_in[bi][:], lsum_sb[:, ca:cb])
            nc.gpsimd.collective_compute(
                "AllReduce", ALU.add,
                replica_groups=[list(range(n_cores))],
                ins=[ccs_in[bi][:].opt()], outs=[ccs_out[bi][:].opt()])
            nc.gpsimd.dma_start(sg_sb[:, ca:cb], ccs_out[bi][:])
            nc.scalar.activation(logz_sb[:, ca:cb], sg_sb[:, ca:cb], AF.Ln)
            nc.vector.tensor_scalar(nlz_sb[:, ca:cb], logz_sb[:, ca:cb],
                                    -1.0, -1.0, op0=ALU.mult, op1=ALU.add)

        def fin_half(mi, half):
            """out = logits - (1 + lnS), reloading logits from DRAM."""
            r0 = mi * 128
            rows = min(128, TB - r0)
            hs = slice(half * HALF, (half + 1) * HALF)
            ob = pools["fin"].tile([128, HALF], BF16, tag="ob")
            nc.sync.dma_start(ob[:rows, :], logits_d[mi, :rows, hs])
            if (2 * mi + half) % 2 == 0:
                nc.vector.tensor_scalar(
                    ob[:rows, :], ob[:rows, :],
                    nlz_sb[:rows, mi:mi + 1], None, op0=ALU.add)
            else:
                nc.scalar.activation(ob[:rows, :], ob[:rows, :],
                                     AF.Identity,
                                     bias=nlz_sb[:rows, mi:mi + 1])
            t0 = mi * 4
            for tl in range(rows // B):
                nc.gpsimd.dma_start(out[:, t0 + tl, hs],
                                    ob[tl * B:(tl + 1) * B, :])

        def d1_sub(s0, nt, u):
            """Attention sub-unit: j-pairs 2u, 2u+1 of a step block.

            Block covers h slots s0..s0+nt-1 -> out-t s0-1..s0+nt-2.
            scores -> softmax -> attn.T -> ctx.T -> Gc (fp8 x16).
            """
            for j in (2 * u, 2 * u + 1):
                scp = pools["pps"].tile([128, 512], F32, tag="zx")
                for hf in range(2):
                    b = 2 * j + hf
                    po = 64 * hf
                    for k in range(4):
                        nc.tensor.matmul(
                            scp[po:po + nt, :64],
                            Hk[:, k, s0:s0 + nt, b],
                            epT_sb[:, k * BL + b * L: k * BL + (b + 1) * L],
                            start=(k == 0), stop=(k == 3))
                att_f = d1w.tile([128, 64], F32, tag="af")
                attb = d1w.tile([128, 64], BF16, tag="ab")
                for hf in range(2):
                    po = 64 * hf
                    nmx = d1s.tile([128, 1], F32, tag="nm")
                    nc.vector.tensor_reduce(nmx[po:po + nt, :],
                                            scp[po:po + nt, :64],
                                            axis=AX.X, op=ALU.max,
                                            negate=True)
                    ssum = d1s.tile([128, 1], F32, tag="ss")
                    nc.scalar.activation(att_f[po:po + nt, :],
                                         scp[po:po + nt, :64],
                                         AF.Exp, bias=nmx[po:po + nt, :],
                                         accum_out=ssum[po:po + nt, :])
                    rcp = d1s.tile([128, 1], F32, tag="rc")
                    nc.vector.reciprocal(rcp[po:po + nt, :],
                                         ssum[po:po + nt, :])
                    nc.vector.tensor_scalar_mul(attb[po:po + nt, :],
                                                att_f[po:po + nt, :],
                                                rcp[po:po + nt, :])
                atT = d1w.tile([128, 16], BF16, tag="atT")
                for hf in range(2):
                    po = 64 * hf
                    tpp = pools["tpsA"].tile([128, 128], BF16, tag="tpb")
                    nc.tensor.transpose(tpp[po:po + L, :nt],
                                        attb[po:po + nt, :L],
                                        idt[po:po + nt, po:po + nt])
                    nc.vector.tensor_copy(atT[po:po + L, :nt],
                                          tpp[po:po + L, :nt])
                for hf in range(2):
                    b = 2 * j + hf
                    po = 64 * hf
                    for mu in range(4):
                        ctp = pools["pps"].tile([128, 512], F32, tag="zx")
                        nc.tensor.matmul(
                            ctp[:, :nt],
                            enc_sb[po:po + L,
                                   j * U + mu * 128: j * U + (mu + 1) * 128],
                            atT[po:po + L, :nt],
                            start=True, stop=True)
                        nc.vector.tensor_scalar_mul(
                            Gck[:, mu, s0 - 1: s0 - 1 + nt, b],
                            ctp[:, :nt], SA)

        # ================================================================
        with ExitStack() as pscope:
            psb = pscope.enter_context(tc.tile_pool(name="p_sbuf", bufs=2))
            pps = pscope.enter_context(
                tc.tile_pool(name="p_psum", bufs=1, space="PSUM"))
            pools["pps"] = pps
            rzp = pscope.enter_context(
                tc.tile_pool(name="r_zps", bufs=1, space="PSUM"))
            pools["fps"] = pscope.enter_context(
                tc.tile_pool(name="fc_psum", bufs=1, space="PSUM"))
            pools["tpsA"] = pscope.enter_context(
                tc.tile_pool(name="tpa_ps", bufs=1, space="PSUM"))
            rga = pscope.enter_context(tc.tile_pool(name="r_gate", bufs=2))

            # pools released mid-R to free SBUF (closed at steps 26 / 18)
            zscope = ExitStack()
            zwp = zscope.enter_context(tc.tile_pool(name="zwp", bufs=1))
            # x.T bf16: col = j*TB + row  (j = E-chunk), x SX
            xt_sb = zwp.tile([128, 2 * TB], BF16)
            xt_j = xt_sb[:].rearrange("p (j r) -> p j r", j=2)
            wx_sb = zwp.tile([128, 16 * 2 * 128], BF16)
            wx_k = wx_sb[:].rearrange("p (mi j m) -> p mi j m", mi=16, j=2)
            if lstm_bias:
                wxb_sb = zwp.tile([1, G4U], BF16)
            escope = ExitStack()
            ewp = escope.enter_context(tc.tile_pool(name="ewp", bufs=1))
            enct_sb = ewp.tile([128, 4 * BL], BF16)
            wa_sb = ewp.tile([128, 4 * U], BF16)

            def emit_zx_unit(nb, mi):
                """One (t-block, m-chunk) unit of Zx.T = W_x.T@X.T (+b)."""
                t0 = nb * 16
                tn = min(16, T - t0)
                ncols = tn * B
                zps = pps.tile([128, 512], F32, tag="zx")
                for k in range(2):
                    nc.tensor.matmul(
                        zps[:, :ncols],
                        wx_k[:, mi, k],
                        xt_j[:, k, t0 * B: t0 * B + ncols],
                        start=(k == 0), stop=(k == 1) and not lstm_bias)
                if lstm_bias:
                    nc.tensor.matmul(zps[:, :ncols],
                                     wxb_sb[:1, mi * 128:(mi + 1) * 128],
                                     ones_bf[:1, :ncols],
                                     start=False, stop=True,
                                     skip_group_check=True)
                # stage into SBUF as fp8 x SZX
                nc.vector.tensor_scalar_mul(
                    zxt_blk[:, mi, t0:t0 + tn, :],
                    zps[:, :ncols].rearrange("p (t b) -> p t b", b=B),
                    SZX / (SWX * SX))

            def emit_ep_unit(un):
                """ep.T = (enc @ Wa/2).T, unit (mu, nb)."""
                mu, nb = un // 4, un % 4
                eps_ = pps.tile([128, 512], F32, tag="zx")
                for k in range(4):
                    nc.tensor.matmul(
                        eps_[:, :],
                        wa_sb[:, k * U + mu * 128:
                              k * U + (mu + 1) * 128],
                        enct_sb[:, k * BL + nb * 512:
                                (k * BL + (nb + 1) * 512)],
                        start=(k == 0), stop=(k == 3))
                nc.vector.tensor_copy(
                    epT_sb[:, mu * BL + nb * 512:
                           mu * BL + (nb + 1) * 512],
                    eps_[:])

            # ============ phase P: minimal preamble ============
            def emit_gather(i):
                r0 = i * 128
                rows = min(128, TB - r0)
                ix = psb.tile([128, 1], I32, tag="ix")
                nc.sync.dma_start(ix[:rows, :], tidx[r0:r0 + rows, :])
                xg = psb.tile([128, E], BF16, tag="xg")
                nc.gpsimd.indirect_dma_start(
                    out=xg[:rows, :], out_offset=None,
                    in_=emb_bf[:],
                    in_offset=bass.IndirectOffsetOnAxis(
                        ap=ix[:rows, :1], axis=0),
                )
                for cc in range(2):
                    tp = pools["tpsA"].tile([128, 128], BF16, tag="tpb")
                    nc.tensor.transpose(
                        tp[:, :rows],
                        xg[:rows, cc * 128:(cc + 1) * 128],
                        idt[:rows, :rows])
                    nc.vector.tensor_scalar_mul(
                        xt_j[:, cc, r0: r0 + rows],
                        tp[:, :rows], SX)

            # big packed weight loads first (spread across DGE queues)
            nc.sync.dma_start(wx_sb[:], wx_q[:])
            nc.scalar.dma_start(wh_sb[:], wh_q[:])
            if lstm_bias:
                nc.sync.dma_start(wxb_sb[:], wxb_in[:])

            for i in range(4):
                emit_gather(i)

            # h0/c0 init: H = 2h, H8 = SA*h, cT = 2c
            hc_sb = psb.tile([B, U], F32, tag="hc")
            nc.sync.dma_start(hc_sb[:, :], h0[:, :])
            cc_sb = psb.tile([B, U], F32, tag="hc2")
            nc.sync.dma_start(cc_sb[:, :], c0[:, :])
            for k in range(4):
                tp = pps.tile([128, 512], F32, tag="zx")
                nc.tensor.transpose(
                    tp[:, :B], hc_sb[:B, k * 128:(k + 1) * 128],
                    idtf[:B, :B])
                nc.vector.tensor_scalar_mul(Hk[:, k, 0, :],
                                            tp[:, :B], 2.0)
                nc.vector.tensor_scalar_mul(H8k[:, k, 0, :],
                                            tp[:, :B], SA)
                tp2 = pps.tile([128, 512], F32, tag="zx")
                nc.tensor.transpose(
                    tp2[:, :B], cc_sb[:B, k * 128:(k + 1) * 128],
                    idtf[:B, :B])
                nc.vector.tensor_scalar_mul(
                    cT[:, k * B:(k + 1) * B], tp2[:, :B], 2.0)

            for mi in range(16):
                emit_zx_unit(0, mi)
            for i in range(4, 16):
                emit_gather(i)

            # remaining loads (all overlap the early recurrence)
            nc.scalar.dma_start(wfc_sb[:], wfc_q[:])
            if fc_bias:
                nc.sync.dma_start(bfc_row[:, :], bfc_in[:, :])
            nc.scalar.dma_start(enct_sb[:], enct_bf[:])
            nc.sync.dma_start(wa_sb[:], wa_bf[:])
            nc.sync.dma_start(enc_sb[:], enc_bf[:])

            # ============ phase R: the master pipeline ============
            if True:
                # ---- interleave schedule: step -> work units ----
                sched = defaultdict(list)
                for t in range(2, 10):      # Zx block 1 + ep 0..7
                    g = t - 2
                    sched[t] += [lambda m=2 * g: emit_zx_unit(1, m),
                                 lambda m=2 * g + 1: emit_zx_unit(1, m),
                                 lambda un=g: emit_ep_unit(un)]
                for t in range(10, 18):     # Zx block 2 + ep 8..15
                    g = t - 10
                    sched[t] += [lambda m=2 * g: emit_zx_unit(2, m),
                                 lambda m=2 * g + 1: emit_zx_unit(2, m),
                                 lambda un=8 + g: emit_ep_unit(un)]
                sched[18] += [escope.close]
                for t in range(18, 26):     # Zx block 3 + D1 block A
                    g = t - 18
                    sched[t] += [lambda m=2 * g: emit_zx_unit(3, m),
                                 lambda m=2 * g + 1: emit_zx_unit(3, m),
                                 lambda u=g: d1_sub(1, 16, u)]

                def open_fc_pools():
                    zscope.close()
                    pools["lgp"] = pscope.enter_context(
                        tc.tile_pool(name="lgp", bufs=2))
                    pools["scr"] = pscope.enter_context(
                        tc.tile_pool(name="scr", bufs=1))
                    pools["fin"] = pscope.enter_context(
                        tc.tile_pool(name="fin", bufs=3))
                sched[26] += [open_fc_pools]
                for t in range(26, 34):     # FC chunks 0..3
                    g = t - 26
                    sched[t] += [lambda mi=g // 2, hf=g % 2: fc_half(mi, hf)]
                for t in range(34, 42):     # D1 block B
                    g = t - 34
                    sched[t] += [lambda u=g: d1_sub(17, 16, u)]
                for t in range(42, 50):     # FC chunks 4..7
                    g = t - 42
                    sched[t] += [lambda mi=4 + g // 2, hf=g % 2:
                                 fc_half(mi, hf)]
                sched[50] += [lambda: ar_batch(0)]
                for t in range(50, 56):     # D1 block C (compressed to 6)
                    g = t - 50
                    sched[t] += [lambda u=g: d1_sub(33, 16, u)]
                    if g >= 4:
                        sched[t] += [lambda u=g + 2: d1_sub(33, 16, u)]
                for t in range(55, 63):     # fins chunks 0..3 (batch 0)
                    g = t - 55
                    sched[t] += [lambda mi=g // 2, hf=g % 2:
                                 fin_half(mi, hf)]
                for t in range(56, 60):     # D1 block D (t 48..55)
                    g = t - 56
                    sched[t] += [lambda u=2 * g: d1_sub(49, 8, u),
                                 lambda u=2 * g + 1: d1_sub(49, 8, u)]
                for t in range(56, 63):     # FC chunks 8..11 (7 of 8)
                    g = t - 56
                    sched[t] += [lambda mi=8 + g // 2, hf=g % 2:
                                 fc_half(mi, hf)]

                c_prev = cT
                for t in range(T):
                    # gate order [g, i, f, o] (host-permuted);
                    # sigmoid(z) = (1+tanh(z/2))/2, folded.
                    gate = {}
                    for gi in range(4):
                        zp = rzp.tile([128, 128], F32, tag=f"z{gi}",
                                      name=f"z{gi}")
                        for m2 in range(4):
                            mi = gi * 4 + m2
                            for kk in range(2):
                                nc.tensor.matmul(
                                    zp[:, m2 * B:(m2 + 1) * B],
                                    wh_k[:, mi, kk],
                                    H8k[:, 2 * kk:2 * kk + 2, t, :],
                                    start=(kk == 0), stop=(kk == 1),
                                    perf_mode=PM.DoubleRow)
                        sl = slice(gi * 128, (gi + 1) * 128)
                        # z += zx*(SZ/SZX) (staged fp8) in-place in PSUM
                        nc.vector.scalar_tensor_tensor(
                            zp[:, :], zxt_t[:, t, sl], SZ / SZX,
                            zp[:, :], op0=ALU.mult, op1=ALU.add)
                        gt = rga.tile([128, 128], F32, tag=f"g{gi}",
                                      name=f"g{gi}")
                        nc.scalar.activation(
                            gt[:], zp[:, :], AF.Tanh,
                            scale=(1.0 if gi == 0 else 0.5) / SZ)
                        gate[gi] = gt
                        if gi == 1:
                            # Bv = (1+ti)*tg  (= 2*i*g)
                            ig = rga.tile([128, 128], F32, tag="ig")
                            nc.vector.scalar_tensor_tensor(
                                ig[:], gate[1][:], 1.0, gate[0][:],
                                op0=ALU.add, op1=ALU.mult)
                        elif gi == 2:
                            # A = (1+tf)*st ; st_new = A/2 + Bv
                            fc_ = rga.tile([128, 128], F32, tag="fc")
                            nc.vector.scalar_tensor_tensor(
                                fc_[:], gate[2][:], 1.0, c_prev[:],
                                op0=ALU.add, op1=ALU.mult)
                            c_new = rga.tile([128, 128], F32, tag="cn")
                            nc.vector.scalar_tensor_tensor(
                                c_new[:], fc_[:], 0.5, ig[:],
                                op0=ALU.mult, op1=ALU.add)
                            tc_ = rga.tile([128, 128], F32, tag="tc")
                            nc.scalar.activation(tc_[:], c_new[:],
                                                 AF.Tanh, scale=0.5)
                    # H8(t+1) = SA*h = ((SA/2)(1+to)) * tanh(c)  [chain]
                    # H(t+1)  = (1+to)*tanh(c) (= 2h)            [lagged ok]
                    g3p = rga.tile([128, 128], F32, tag="g3p")
                    nc.vector.tensor_scalar(g3p[:], gate[3][:],
                                            SA / 2.0, SA / 2.0,
                                            op0=ALU.mult, op1=ALU.add)
                    nc.vector.tensor_tensor(
                        H8k[:, :, t + 1, :],
                        g3p[:].rearrange("p (k b) -> p k b", k=4),
                        tc_[:].rearrange("p (k b) -> p k b", k=4),
                        op=ALU.mult)
                    nc.vector.scalar_tensor_tensor(
                        Hk[:, :, t + 1, :],
                        gate[3][:].rearrange("p (k b) -> p k b", k=4),
                        1.0,
                        tc_[:].rearrange("p (k b) -> p k b", k=4),
                        op0=ALU.add, op1=ALU.mult)
                    c_prev = c_new
                    for unit in sched[t]:
                        unit()

                # =================== tail ===================
                fc_half(11, 1)
                fins_a = [(4, 0), (4, 1), (5, 0), (5, 1)]
                for u in range(8):          # D1 block E (t 56..62)
                    d1_sub(57, 7, u)
                    if u % 2 == 1:
                        fin_half(*fins_a[u // 2])
                fins_b = [(6, 0), (6, 1), (7, 0), (7, 1)]
                for g in range(8):          # FC chunks 12..15
                    fc_half(12 + g // 2, g % 2)
                    if g % 2 == 1:
                        fin_half(*fins_b[g // 2])
                ar_batch(1)
                for mi in range(8, 16):
                    for hf in range(2):
                        fin_half(mi, hf)

    nc.compile()
    return nc


def _bf(x):
    return np.ascontiguousarray(
        np.asarray(x, np.float32).astype(ml_dtypes.bfloat16))


def _q8(x, scale):
    y = np.asarray(x, np.float32) * scale
    y = np.clip(y, -240.0, 240.0)
    return np.ascontiguousarray(y.astype(ml_dtypes.float8_e4m3))


def prep_inputs(target, encoder_outputs, enc_h0, enc_c0, emb, W_x, W_h,
                b_lstm, Wa, W_fc, b_fc, n_cores=NC):
    """Host-side layout prep + per-core sharding."""
    tgt = np.asarray(target).astype(np.int32)
    tidx = np.ascontiguousarray(tgt[:, :T].T.reshape(TB, 1))  # t-major rows
    enc = np.asarray(encoder_outputs, np.float32)
    # permute gate columns [i,f,g,o] -> [g,i,f,o]
    gperm = np.r_[2 * U:3 * U, 0:U, U:2 * U, 3 * U:4 * U]
    Wxp = np.asarray(W_x, np.float32)[:, gperm]
    # wx fp8 packed [128, (mi, j, m)]
    wx_pack = Wxp.reshape(2, 128, 16, 128).transpose(1, 2, 0, 3) \
        .reshape(128, 16 * 2 * 128)
    # W_h fp8 packed [128, (mi, kk, j, m)]
    Whp = np.asarray(W_h, np.float32)[:, gperm]          # [512, 2048]
    wh = Whp.reshape(4, 128, 16, 128)                    # [kt, p, mi, m]
    wh = wh.reshape(2, 2, 128, 16, 128)                  # [kk, j, p, mi, m]
    wh = wh.transpose(2, 3, 0, 1, 4).reshape(128, 16 * 2 * 2 * 128)
    # enc packed: enc_sb[0:64, j*U:] = enc[2j], [64:128] = enc[2j+1]
    enc_pack = np.empty((128, 16 * U), np.float32)
    for j in range(16):
        enc_pack[0:64, j * U:(j + 1) * U] = enc[2 * j]
        enc_pack[64:128, j * U:(j + 1) * U] = enc[2 * j + 1]
    # enct packed [128, (k, b, l)]
    enct = enc.transpose(2, 0, 1).reshape(U, BL)         # [U, (b,l)]
    enct_pack = enct.reshape(4, 128, BL).transpose(1, 0, 2).reshape(128, 4 * BL)
    # wa packed [128, (k, m)] with Wa/2 (absorbs H=2h in scores)
    wa = np.asarray(Wa, np.float32) * 0.5
    wa_pack = wa.reshape(4, 128, U).transpose(1, 0, 2).reshape(128, 4 * U)
    b_lstm = np.asarray(b_lstm, np.float32)
    lstm_bias = bool(np.any(b_lstm))
    common = {
        "tidx": tidx,
        "emb_bf": _bf(emb),
        "wx_q": _bf(wx_pack * SWX),
        "wh_q": _q8(wh, SWH),
        "enc_bf": _bf(enc_pack),
        "enct_bf": _bf(enct_pack),
        "wa_bf": _bf(wa_pack),
        "h0": np.ascontiguousarray(np.asarray(enc_h0, np.float32)),
        "c0": np.ascontiguousarray(np.asarray(enc_c0, np.float32)),
    }
    if lstm_bias:
        # lands in the zx PSUM, which carries scale SWX*SX
        common["wxb_bf"] = _bf(b_lstm[gperm].reshape(1, G4U) * SWX * SX)
    wfc = np.asarray(W_fc, np.float32)
    bfc = np.asarray(b_fc, np.float32)
    fc_bias = bool(np.any(bfc))
    in_maps = []
    for c in range(n_cores):
        m = dict(common)
        wshard = wfc[:, c * VS:(c + 1) * VS]             # [1024, VS]
        wsh = wshard.reshape(4, 2, 128, VS)              # [j, kk, p, v]
        wsh = wsh.transpose(2, 0, 3, 1).reshape(128, 4 * VS * 2)
        m["wfc_q"] = _q8(wsh, SW)
        if fc_bias:
            m["bfc_bf"] = _bf(bfc[c * VS:(c + 1) * VS].reshape(1, VS) * SP)
        in_maps.append(m)
    return in_maps, fc_bias, lstm_bias


def kernel(**inputs):
    in_maps, fc_bias, lstm_bias = prep_inputs(**inputs, n_cores=NC)
    key = ("nc", fc_bias, lstm_bias)
    if key not in _CACHE:
        _CACHE[key] = build(NC, fc_bias=fc_bias, lstm_bias=lstm_bias)
        _CACHE["nc"] = _CACHE[key]
    nc = _CACHE[key]
    res = run_bass_kernel_spmd(nc, in_maps, list(range(NC)))
    shards = [np.asarray(res.results[c]["out"]).astype(np.float32)
              for c in range(NC)]
    return np.concatenate(shards, axis=-1)


# revision 41
# speedup vs baseline: 1.3560x; 1.3560x over previous
"""Trainium2 Bass kernel for nn_Decoder_43336220016932.

Luong-attention LSTM decoder with teacher forcing:
  out[b,t,:] = log_softmax(tanh([ctx_t, h_t] @ W_fc + b_fc))

v2 strategy (8 NeuronCores):
  - Vocab-sharded tensor parallel: core i owns W_fc[:, i*4000:(i+1)*4000]
    resident in SBUF as fp8 (x256), k-pair INTERLEAVED so the DoubleRow
    moving reads are contiguous. The serial LSTM recurrence is replicated
    on all cores; attention / FC / finalize are pipelined into it.
  - The recurrence h@W_h runs in fp8 DoubleRow (W_h x1024 fp8 stationary,
    fp8 h-state H8 = 16*h as moving): 32 LDW+MM pairs per step instead of
    64 bf16 pairs. PSUM holds z*16384; x@W_x (+b_lstm) is precomputed in
    blocks (bf16, x16384) into SBUF and added into PSUM in-place by DVE.
  - All sigmoids as tanh (sigmoid(x)=(1+tanh(x/2))/2) with affine factors
    folded (c kept as 2c, h as 2h); one ACT table set, no reloads.
  - State transposed ([U -> partitions, B -> free]); H8 (fp8 16h) feeds
    both the recurrence matmul and the FC; H (bf16 2h) feeds attention.
  - logits are tanh-bounded: sumexp uses exp(x-1); log_softmax needs one
    AllReduce(add) per batch of 8 chunks and no max pass.
  - Finalize (logits - logZ) reloads logits from DRAM, spread across
    Vector/Scalar, writes bf16 output via the gpsimd DMA queue.
"""
from collections import defaultdict
from contextlib import ExitStack

import numpy as np
import ml_dtypes

import concourse.bass as bass
import concourse.tile as tile
from concourse import bacc, mybir
from concourse.bass_utils import run_bass_kernel_spmd
from concourse.masks import make_identity

B, S, L, U, E, V = 32, 64, 64, 512, 256, 32000
T = S - 1                  # 63 decode steps
NC = 8                     # cores
VS = V // NC               # 4000 vocab shard per core
TB = T * B                 # 2016 (t, b) rows, t-major
G4U = 4 * U                # 2048
BL = B * L                 # 2048
SW = 256.0                 # fp8 scale on W_fc
SA = 16.0                  # fp8 scale on ctx / h
SP = SW * SA               # product scale on logits in PSUM
SWH = 1024.0               # fp8 scale on W_h
SZ = SWH * SA              # scale of z in PSUM
SZX = 2048.0               # fp8 scale on staged zx
SX = 128.0                 # fp8 scale on gathered x
SWX = 1024.0               # fp8 scale on W_x
AF = mybir.ActivationFunctionType
ALU = mybir.AluOpType
AX = mybir.AxisListType
PM = mybir.MatmulPerfMode
F32 = mybir.dt.float32
BF16 = mybir.dt.bfloat16
FP8 = mybir.dt.float8e4
I32 = mybir.dt.int32
HALF = VS // 2             # 2000
QSL = [(0, 512), (512, 512), (1024, 512), (1536, 464)]

_CACHE = {}


def build(n_cores=NC, fc_bias=False, lstm_bias=False):
    """Build the SPMD Bass program (same program on every core)."""
    nc = bacc.Bacc("TRN2", target_bir_lowering=False, debug=False,
                   num_devices=n_cores)

    # ---- external I/O ----
    tidx = nc.dram_tensor("tidx", [TB, 1], I32, kind="ExternalInput").ap()
    emb_bf = nc.dram_tensor("emb_bf", [V, E], BF16, kind="ExternalInput").ap()
    # W_x fp8 [128, (mi=16, j=2, m=128)] (x SWX)
    wx_q = nc.dram_tensor("wx_q", [128, 16 * 2 * 128], FP8,
                          kind="ExternalInput").ap()
    wh_q = nc.dram_tensor("wh_q", [128, 16 * 2 * 2 * 128], FP8,
                          kind="ExternalInput").ap()
    enc_bf = nc.dram_tensor("enc_bf", [128, 16 * U], BF16,
                            kind="ExternalInput").ap()
    enct_bf = nc.dram_tensor("enct_bf", [128, 4 * BL], BF16,
                             kind="ExternalInput").ap()
    wa_bf = nc.dram_tensor("wa_bf", [128, 4 * U], BF16,
                           kind="ExternalInput").ap()
    h0 = nc.dram_tensor("h0", [B, U], F32, kind="ExternalInput").ap()
    c0 = nc.dram_tensor("c0", [B, U], F32, kind="ExternalInput").ap()
    # W_fc fp8, [128, (j=4, v=VS, kk=2)] interleaved k-pairs
    wfc_q = nc.dram_tensor("wfc_q", [128, 4 * VS * 2], FP8,
                           kind="ExternalInput").ap()
    if lstm_bias:
        wxb_in = nc.dram_tensor("wxb_bf", [1, G4U], BF16,
                                kind="ExternalInput").ap()
    if fc_bias:
        bfc_in = nc.dram_tensor("bfc_bf", [1, VS], BF16,
                                kind="ExternalInput").ap()
    out = nc.dram_tensor("out", [B, T, VS], BF16, kind="ExternalOutput").ap()

    with tile.TileContext(nc) as tc, ExitStack() as perm:
        # ---------------- permanent pools ----------------
        konst = perm.enter_context(tc.tile_pool(name="konst", bufs=1))
        wpool = perm.enter_context(tc.tile_pool(name="wpool", bufs=1))
        hpool = perm.enter_context(tc.tile_pool(name="hpool", bufs=1))
        dram = perm.enter_context(tc.tile_pool(name="dram", bufs=1, space="DRAM"))
        stats = perm.enter_context(tc.tile_pool(name="stats", bufs=1))

        idt = konst.tile([128, 128], BF16)
        make_identity(nc, idt[:])
        negone = konst.tile([128, 1], F32)
        nc.vector.memset(negone[:], -1.0)
        idtf = konst.tile([128, 128], F32)
        make_identity(nc, idtf[:])
        ones_bf = konst.tile([1, 512], BF16)
        nc.vector.memset(ones_bf[:], 1.0)
        # fp8 identity x (SZ/SZX): folds the zx rescale into a PE-only add
        idt8 = konst.tile([128, 128], FP8)
        nc.vector.tensor_scalar_mul(idt8[:], idtf[:], SZ / SZX)

        # W_fc fp8 [p, (j, v, kk)]: per j-group the k-pair is interleaved
        wfc_sb = wpool.tile([128, 4 * VS * 2], FP8)
        wfc_k = wfc_sb[:].rearrange("p (j v k) -> p j k v", j=4, k=2)
        # W_h fp8 [p, (mi, kk, j, m)]
        wh_sb = wpool.tile([128, 16 * 2 * 2 * 128], FP8)
        wh_k = wh_sb[:].rearrange("p (mi kk j m) -> p mi kk j m",
                                  mi=16, kk=2, j=2)
        if fc_bias:
            bfc_row = wpool.tile([1, VS], BF16)

        # H: (2h).T history (bf16, attention scores); H8: fp8 x16 h
        # (recurrence + FC). col = k*2048 + slot*32 + b
        H = hpool.tile([128, 4 * 64 * B], BF16)
        Hk = H[:].rearrange("p (k s b) -> p k s b", k=4, s=64)
        H8 = hpool.tile([128, 4 * 64 * B], FP8)
        H8k = H8[:].rearrange("p (k s b) -> p k s b", k=4, s=64)
        H8r = H8[:].rearrange("p (k c) -> p k c", k=4)
        # G_ctx: ctx.T fp8 x16, col = k*2016 + t*32 + b
        Gc = hpool.tile([128, 4 * TB], FP8)
        Gck = Gc[:].rearrange("p (k t b) -> p k t b", k=4, t=T)
        Gcr = Gc[:].rearrange("p (k r) -> p k r", k=4)
        cT = hpool.tile([128, 128], F32)     # (2c).T state, col = k*32+b

        # Zx staged in SBUF: [p, (t, mi, b)] fp8, values x SZX
        zxt_sb = hpool.tile([128, T * 512], FP8)
        zxt_t = zxt_sb[:].rearrange("p (t c) -> p t c", t=T)
        zxt_blk = zxt_sb[:].rearrange("p (t m b) -> p m t b", t=T, m=16)

        # per-row ((t,b) grouped [128 x 16]) log-softmax stats.
        lsum_sb = stats.tile([128, 16], F32)   # local sum exp(x - 1)
        sg_sb = stats.tile([128, 16], F32)     # global sum
        logz_sb = stats.tile([128, 16], F32)   # ln(global sum)
        nlz_sb = stats.tile([128, 16], F32)    # -(1 + ln(global sum))
        nc.vector.memset(lsum_sb[:], 1.0)

        # DRAM scratch
        logits_d = dram.tile([16, 128, VS], BF16)    # tanh'd logits
        ccs_in = [dram.tile([128, 8], F32, name=f"cci{i}") for i in range(2)]
        ccs_out = [dram.tile([128, 8], F32, name=f"cco{i}") for i in range(2)]

        mwp = perm.enter_context(tc.tile_pool(name="midw", bufs=1))
        epT_sb = mwp.tile([128, 4 * BL], BF16)      # (enc @ Wa/2).T
        enc_sb = mwp.tile([128, 16 * U], BF16)      # 2 b per 64-row group

        d1w = perm.enter_context(tc.tile_pool(name="d1w", bufs=3))
        d1s = perm.enter_context(tc.tile_pool(name="d1s", bufs=3))
        sst = perm.enter_context(tc.tile_pool(name="sst", bufs=4))

        lg_tiles = {}
        ac_tiles = {}

        # =========== schedulable work units (emitted into R) ===========
        pools = {}

        def fc_half(mi, half):
            """FC chunk mi, vocab half (2 quarter-psum tiles):
            fp8 DoubleRow + tanh + exp."""
            r0 = mi * 128
            rows = min(128, TB - r0)
            if half == 0:
                lg_tiles[mi] = pools["lgp"].tile([128, VS], BF16, tag="lg",
                                                 name="lg")
            lg = lg_tiles[mi]
            acqs = []
            for q in range(2):
                base = half * HALF + q * 1000
                fcp = pools["fps"].tile([128, 1000], F32, tag="fc")
                for j in range(4):
                    lhs = Gcr[:, 2 * j: 2 * j + 2, r0:r0 + rows] if j < 2 \
                        else H8r[:, 2 * (j - 2): 2 * (j - 2) + 2,
                                 B + r0: B + r0 + rows]
                    for off, w in [(0, 512), (512, 488)]:
                        nc.tensor.matmul(
                            fcp[:rows, off:off + w], lhs,
                            wfc_k[:, j, :, base + off: base + off + w],
                            start=(j == 0), stop=(j == 3) and not fc_bias,
                            perf_mode=PM.DoubleRow)
                if fc_bias:
                    for off, w in [(0, 512), (512, 488)]:
                        nc.tensor.matmul(
                            fcp[:rows, off:off + w],
                            ones_bf[:1, :rows],
                            bfc_row[:1, base + off: base + off + w],
                            start=False, stop=True,
                            skip_group_check=True)
                qs = slice(base, base + 1000)
                nc.scalar.activation(lg[:rows, qs], fcp[:rows, :], AF.Tanh,
                                     scale=1.0 / SP)
                sc_ = pools["scr"].tile([128, 1000], BF16, tag="sc")
                acx = sst.tile([128, 1], F32, tag="ac")
                nc.scalar.activation(sc_[:rows, :], lg[:rows, qs], AF.Exp,
                                     bias=negone[:rows, :],
                                     accum_out=acx[:rows, :])
                nc.sync.dma_start(logits_d[mi, :rows, qs], lg[:rows, qs])
                acqs.append(acx)
            hsum = sst.tile([128, 1], F32, tag="hs")
            nc.vector.tensor_add(hsum[:rows, :], acqs[0][:rows, :],
                                 acqs[1][:rows, :])
            if half == 0:
                ac_tiles[mi] = hsum
            else:
                nc.vector.tensor_add(lsum_sb[:rows, mi:mi + 1],
                                     ac_tiles[mi][:rows, :], hsum[:rows, :])

        def ar_batch(bi):
            """AllReduce batch bi's sumexp; nlz = -(1 + ln S)."""
            ca, cb = 8 * bi, 8 * bi + 8
            nc.sync.dma_start(ccs_in[bi][:], lsum_sb[:, ca:cb])
            nc.gpsimd.collective_compute(
                "AllReduce", ALU.add,
                replica_groups=[list(range(n_cores))],
                ins=[ccs_in[bi][:].opt()], outs=[ccs_out[bi][:].opt()])
            nc.gpsimd.dma_start(sg_sb[:, ca:cb], ccs_out[bi][:])
            nc.scalar.activation(logz_sb[:, ca:cb], sg_sb[:, ca:cb], AF.Ln)
            nc.vector.tensor_scalar(nlz_sb[:, ca:cb], logz_sb[:, ca:cb],
                                    -1.0, -1.0, op0=ALU.mult, op1=ALU.add)

        def fin_half(mi, half):
            """out = logits - (1 + lnS), reloading logits from DRAM."""
            r0 = mi * 128
            rows = min(128, TB - r0)
            hs = slice(half * HALF, (half + 1) * HALF)
            ob = pools["fin"].tile([128, HALF], BF16, tag="ob")
            nc.sync.dma_start(ob[:rows, :], logits_d[mi, :rows, hs])
            if (2 * mi + half) % 2 == 0:
                nc.vector.tensor_scalar(
                    ob[:rows, :], ob[:rows, :],
                    nlz_sb[:rows, mi:mi + 1], None, op0=ALU.add)
            else:
                nc.scalar.activation(ob[:rows, :], ob[:rows, :],
                                     AF.Identity,
                                     bias=nlz_sb[:rows, mi:mi + 1])
            t0 = mi * 4
            for tl in range(rows // B):
                nc.gpsimd.dma_start(out[:, t0 + tl, hs],
                                    ob[tl * B:(tl + 1) * B, :])

        def d1_sub(s0, nt, u):
            """Attention sub-unit: j-pairs 2u, 2u+1 of a step block.

            Block covers h slots s0..s0+nt-1 -> out-t s0-1..s0+nt-2.
            scores -> softmax -> attn.T -> ctx.T -> Gc (fp8 x16).
            """
            for j in (2 * u, 2 * u + 1):
                scp = pools["pps"].tile([128, 512], F32, tag="zx")
                for hf in range(2):
                    b = 2 * j + hf
                    po = 64 * hf
                    for k in range(4):
                        nc.tensor.matmul(
                            scp[po:po + nt, :64],
                            Hk[:, k, s0:s0 + nt, b],
                            epT_sb[:, k * BL + b * L: k * BL + (b + 1) * L],
                            start=(k == 0), stop=(k == 3))
                att_f = d1w.tile([128, 64], F32, tag="af")
                attb = d1w.tile([128, 64], BF16, tag="ab")
                for hf in range(2):
                    po = 64 * hf
                    nmx = d1s.tile([128, 1], F32, tag="nm")
                    nc.vector.tensor_reduce(nmx[po:po + nt, :],
                                            scp[po:po + nt, :64],
                                            axis=AX.X, op=ALU.max,
                                            negate=True)
                    ssum = d1s.tile([128, 1], F32, tag="ss")
                    nc.scalar.activation(att_f[po:po + nt, :],
                                         scp[po:po + nt, :64],
                                         AF.Exp, bias=nmx[po:po + nt, :],
                                         accum_out=ssum[po:po + nt, :])
                    rcp = d1s.tile([128, 1], F32, tag="rc")
                    nc.vector.reciprocal(rcp[po:po + nt, :],
                                         ssum[po:po + nt, :])
                    nc.vector.tensor_scalar_mul(attb[po:po + nt, :],
                                                att_f[po:po + nt, :],
                                                rcp[po:po + nt, :])
                atT = d1w.tile([128, 16], BF16, tag="atT")
                for hf in range(2):
                    po = 64 * hf
                    tpp = pools["tpsA"].tile([128, 128], BF16, tag="tpb")
                    nc.tensor.transpose(tpp[po:po + L, :nt],
                                        attb[po:po + nt, :L],
                                        idt[po:po + nt, po:po + nt])
                    nc.vector.tensor_copy(atT[po:po + L, :nt],
                                          tpp[po:po + L, :nt])
                for hf in range(2):
                    b = 2 * j + hf
                    po = 64 * hf
                    ctp = pools["ctp"].tile([128, 4 * 16], F32, tag="ct")
                    for mu in range(4):
                        nc.tensor.matmul(
                            ctp[:, mu * nt:(mu + 1) * nt],
                            enc_sb[po:po + L,
                                   j * U + mu * 128: j * U + (mu + 1) * 128],
                            atT[po:po + L, :nt],
                            start=True, stop=True)
                    nc.vector.tensor_scalar_mul(
                        Gck[:, :, s0 - 1: s0 - 1 + nt, b],
                        ctp[:, :4 * nt].rearrange("p (k tt) -> p k tt", k=4),
                        SA)

        # ================================================================
        with ExitStack() as pscope:
            psb = pscope.enter_context(tc.tile_pool(name="p_sbuf", bufs=2))
            pps = pscope.enter_context(
                tc.tile_pool(name="p_psum", bufs=3, space="PSUM"))
            pools["pps"] = pps
            rzp = pscope.enter_context(
                tc.tile_pool(name="r_zps", bufs=1, space="PSUM"))
            pools["fps"] = pscope.enter_context(
                tc.tile_pool(name="fc_psum", bufs=1, space="PSUM"))
            pools["tpsA"] = pscope.enter_context(
                tc.tile_pool(name="tpa_ps", bufs=1, space="PSUM"))
            pools["ctp"] = pscope.enter_context(
                tc.tile_pool(name="ct_ps", bufs=1, space="PSUM"))
            rga = pscope.enter_context(tc.tile_pool(name="r_gate", bufs=2))

            # pools released mid-R to free SBUF (closed at steps 26 / 18)
            zscope = ExitStack()
            zwp = zscope.enter_context(tc.tile_pool(name="zwp", bufs=1))
            # x.T fp8 interleaved: col = row*2 + j  (j = E-chunk), x SX
            xt_sb = zwp.tile([128, 2 * TB], FP8)
            xt_j = xt_sb[:].rearrange("p (r j) -> p j r", j=2)
            wx_sb = zwp.tile([128, 16 * 2 * 128], FP8)
            wx_k = wx_sb[:].rearrange("p (mi j m) -> p mi j m", mi=16, j=2)
            if lstm_bias:
                wxb_sb = zwp.tile([1, G4U], BF16)
            escope = ExitStack()
            ewp = escope.enter_context(tc.tile_pool(name="ewp", bufs=1))
            enct_sb = ewp.tile([128, 4 * BL], BF16)
            wa_sb = ewp.tile([128, 4 * U], BF16)

            def emit_zx_unit(nb, mi):
                """One (t-block, m-chunk) unit of Zx.T = W_x.T@X.T (+b)."""
                t0 = nb * 16
                tn = min(16, T - t0)
                ncols = tn * B
                zps = pps.tile([128, 512], F32, tag="zx")
                nc.tensor.matmul(
                    zps[:, :ncols],
                    wx_k[:, mi],
                    xt_j[:, :, t0 * B: t0 * B + ncols],
                    start=True, stop=not lstm_bias,
                    perf_mode=PM.DoubleRow)
                if lstm_bias:
                    nc.tensor.matmul(zps[:, :ncols],
                                     wxb_sb[:1, mi * 128:(mi + 1) * 128],
                                     ones_bf[:1, :ncols],
                                     start=False, stop=True,
                                     skip_group_check=True)
                # stage into SBUF as fp8 x SZX
                nc.vector.tensor_scalar_mul(
                    zxt_blk[:, mi, t0:t0 + tn, :],
                    zps[:, :ncols].rearrange("p (t b) -> p t b", b=B),
                    SZX / (SWX * SX))

            def emit_ep_unit(un):
                """ep.T = (enc @ Wa/2).T, unit (mu, nb)."""
                mu, nb = un // 4, un % 4
                eps_ = pps.tile([128, 512], F32, tag="zx")
                for k in range(4):
                    nc.tensor.matmul(
                        eps_[:, :],
                        wa_sb[:, k * U + mu * 128:
                              k * U + (mu + 1) * 128],
                        enct_sb[:, k * BL + nb * 512:
                                (k * BL + (nb + 1) * 512)],
                        start=(k == 0), stop=(k == 3))
                nc.vector.tensor_copy(
                    epT_sb[:, mu * BL + nb * 512:
                           mu * BL + (nb + 1) * 512],
                    eps_[:])

            # ============ phase P: minimal preamble ============
            def emit_gather(i):
                r0 = i * 128
                rows = min(128, TB - r0)
                ix = psb.tile([128, 1], I32, tag="ix")
                nc.sync.dma_start(ix[:rows, :], tidx[r0:r0 + rows, :])
                xg = psb.tile([128, E], BF16, tag="xg")
                nc.gpsimd.indirect_dma_start(
                    out=xg[:rows, :], out_offset=None,
                    in_=emb_bf[:],
                    in_offset=bass.IndirectOffsetOnAxis(
                        ap=ix[:rows, :1], axis=0),
                )
                for cc in range(2):
                    tp = pools["tpsA"].tile([128, 128], BF16, tag="tpb")
                    nc.tensor.transpose(
                        tp[:, :rows],
                        xg[:rows, cc * 128:(cc + 1) * 128],
                        idt[:rows, :rows])
                    nc.vector.tensor_scalar_mul(
                        xt_j[:, cc, r0: r0 + rows],
                        tp[:, :rows], SX)

            # big packed weight loads first (spread across DGE queues)
            nc.sync.dma_start(wx_sb[:], wx_q[:])
            nc.scalar.dma_start(wh_sb[:], wh_q[:])
            if lstm_bias:
                nc.sync.dma_start(wxb_sb[:], wxb_in[:])

            for i in range(4):
                emit_gather(i)

            # h0/c0 init: H = 2h, H8 = SA*h, cT = 2c
            hc_sb = psb.tile([B, U], F32, tag="hc")
            nc.sync.dma_start(hc_sb[:, :], h0[:, :])
            cc_sb = psb.tile([B, U], F32, tag="hc2")
            nc.sync.dma_start(cc_sb[:, :], c0[:, :])
            for k in range(4):
                tp = pps.tile([128, 512], F32, tag="zx")
                nc.tensor.transpose(
                    tp[:, :B], hc_sb[:B, k * 128:(k + 1) * 128],
                    idtf[:B, :B])
                nc.vector.tensor_scalar_mul(Hk[:, k, 0, :],
                                            tp[:, :B], 2.0)
                nc.vector.tensor_scalar_mul(H8k[:, k, 0, :],
                                            tp[:, :B], SA)
                tp2 = pps.tile([128, 512], F32, tag="zx")
                nc.tensor.transpose(
                    tp2[:, :B], cc_sb[:B, k * 128:(k + 1) * 128],
                    idtf[:B, :B])
                nc.vector.tensor_scalar_mul(
                    cT[:, k * B:(k + 1) * B], tp2[:, :B], 2.0)

            for mi in range(16):
                emit_zx_unit(0, mi)
            for i in range(4, 16):
                emit_gather(i)

            # remaining loads (all overlap the early recurrence)
            nc.scalar.dma_start(wfc_sb[:], wfc_q[:])
            if fc_bias:
                nc.sync.dma_start(bfc_row[:, :], bfc_in[:, :])
            nc.scalar.dma_start(enct_sb[:], enct_bf[:])
            nc.sync.dma_start(wa_sb[:], wa_bf[:])
            nc.sync.dma_start(enc_sb[:], enc_bf[:])

            # ============ phase R: the master pipeline ============
            if True:
                # ---- interleave schedule: step -> work units ----
                sched = defaultdict(list)
                for t in range(2, 10):      # Zx block 1 + ep 0..7
                    g = t - 2
                    sched[t] += [lambda m=2 * g: emit_zx_unit(1, m),
                                 lambda m=2 * g + 1: emit_zx_unit(1, m),
                                 lambda un=g: emit_ep_unit(un)]
                for t in range(10, 18):     # Zx block 2 + ep 8..15
                    g = t - 10
                    sched[t] += [lambda m=2 * g: emit_zx_unit(2, m),
                                 lambda m=2 * g + 1: emit_zx_unit(2, m),
                                 lambda un=8 + g: emit_ep_unit(un)]
                sched[18] += [escope.close]
                for t in range(18, 26):     # Zx block 3 + D1 block A
                    g = t - 18
                    sched[t] += [lambda m=2 * g: emit_zx_unit(3, m),
                                 lambda m=2 * g + 1: emit_zx_unit(3, m),
                                 lambda u=g: d1_sub(1, 16, u)]

                def open_fc_pools():
                    zscope.close()
                    pools["lgp"] = pscope.enter_context(
                        tc.tile_pool(name="lgp", bufs=2))
                    pools["scr"] = pscope.enter_context(
                        tc.tile_pool(name="scr", bufs=1))
                    pools["fin"] = pscope.enter_context(
                        tc.tile_pool(name="fin", bufs=3))
                sched[26] += [open_fc_pools]
                for t in range(26, 34):     # FC chunks 0..3
                    g = t - 26
                    sched[t] += [lambda mi=g // 2, hf=g % 2: fc_half(mi, hf)]
                for t in range(34, 42):     # D1 block B
                    g = t - 34
                    sched[t] += [lambda u=g: d1_sub(17, 16, u)]
                for t in range(42, 50):     # FC chunks 4..7
                    g = t - 42
                    sched[t] += [lambda mi=4 + g // 2, hf=g % 2:
                                 fc_half(mi, hf)]
                sched[50] += [lambda: ar_batch(0)]
                for t in range(50, 56):     # D1 block C (compressed to 6)
                    g = t - 50
                    sched[t] += [lambda u=g: d1_sub(33, 16, u)]
                    if g >= 4:
                        sched[t] += [lambda u=g + 2: d1_sub(33, 16, u)]
                for t in range(55, 63):     # fins chunks 0..3 (batch 0)
                    g = t - 55
                    sched[t] += [lambda mi=g // 2, hf=g % 2:
                                 fin_half(mi, hf)]
                for t in range(56, 60):     # D1 block D (t 48..55)
                    g = t - 56
                    sched[t] += [lambda u=2 * g: d1_sub(49, 8, u),
                                 lambda u=2 * g + 1: d1_sub(49, 8, u)]
                for t in range(56, 63):     # FC chunks 8..11 (7 of 8)
                    g = t - 56
                    sched[t] += [lambda mi=8 + g // 2, hf=g % 2:
                                 fc_half(mi, hf)]

                c_prev = cT
                for t in range(T):
                    zps = rzp.tile([128, 512], F32, tag="zt")
                    # gate order [g, i, f, o] (host-permuted);
                    # sigmoid(z) = (1+tanh(z/2))/2, folded.
                    gate = {}
                    for gi in range(4):
                        for m2 in range(4):
                            mi = gi * 4 + m2
                            for kk in range(2):
                                nc.tensor.matmul(
                                    zps[:, mi * B:(mi + 1) * B],
                                    wh_k[:, mi, kk],
                                    H8k[:, 2 * kk:2 * kk + 2, t, :],
                                    start=(kk == 0), stop=(kk == 1),
                                    perf_mode=PM.DoubleRow)
                        sl = slice(gi * 128, (gi + 1) * 128)
                        # zq = z + zx*(SZ/SZX) (staged fp8), into SBUF
                        zq = rga.tile([128, 128], F32, tag=f"q{gi}",
                                      name=f"q{gi}")
                        nc.vector.scalar_tensor_tensor(
                            zq[:], zxt_t[:, t, sl], SZ / SZX,
                            zps[:, sl], op0=ALU.mult, op1=ALU.add)
                        gt = rga.tile([128, 128], F32, tag=f"g{gi}",
                                      name=f"g{gi}")
                        nc.scalar.activation(
                            gt[:], zq[:], AF.Tanh,
                            scale=(1.0 if gi == 0 else 0.5) / SZ)
                        gate[gi] = gt
                        if gi == 1:
                            # Bv = (1+ti)*tg  (= 2*i*g)
                            ig = rga.tile([128, 128], F32, tag="ig")
                            nc.vector.scalar_tensor_tensor(
                                ig[:], gate[1][:], 1.0, gate[0][:],
                                op0=ALU.add, op1=ALU.mult)
                        elif gi == 2:
                            # A = (1+tf)*st ; st_new = A/2 + Bv
                            fc_ = rga.tile([128, 128], F32, tag="fc")
                            nc.vector.scalar_tensor_tensor(
                                fc_[:], gate[2][:], 1.0, c_prev[:],
                                op0=ALU.add, op1=ALU.mult)
                            c_new = rga.tile([128, 128], F32, tag="cn")
                            nc.vector.scalar_tensor_tensor(
                                c_new[:], fc_[:], 0.5, ig[:],
                                op0=ALU.mult, op1=ALU.add)
                            tc_ = rga.tile([128, 128], F32, tag="tc")
                            nc.scalar.activation(tc_[:], c_new[:],
                                                 AF.Tanh, scale=0.5)
                    # H8(t+1) = SA*h = ((SA/2)(1+to)) * tanh(c)  [chain]
                    # H(t+1)  = (1+to)*tanh(c) (= 2h)            [lagged ok]
                    g3p = rga.tile([128, 128], F32, tag="g3p")
                    nc.vector.tensor_scalar(g3p[:], gate[3][:],
                                            SA / 2.0, SA / 2.0,
                                            op0=ALU.mult, op1=ALU.add)
                    nc.vector.tensor_tensor(
                        H8k[:, :, t + 1, :],
                        g3p[:].rearrange("p (k b) -> p k b", k=4),
                        tc_[:].rearrange("p (k b) -> p k b", k=4),
                        op=ALU.mult)
                    nc.vector.scalar_tensor_tensor(
                        Hk[:, :, t + 1, :],
                        gate[3][:].rearrange("p (k b) -> p k b", k=4),
                        1.0,
                        tc_[:].rearrange("p (k b) -> p k b", k=4),
                        op0=ALU.add, op1=ALU.mult)
                    c_prev = c_new
                    for unit in sched[t]:
                        unit()

                # =================== tail ===================
                fc_half(11, 1)
                fins_a = [(4, 0), (4, 1), (5, 0), (5, 1)]
                for u in range(8):          # D1 block E (t 56..62)
                    d1_sub(57, 7, u)
                    if u % 2 == 1:
                        fin_half(*fins_a[u // 2])
                fins_b = [(6, 0), (6, 1), (7, 0), (7, 1)]
                for g in range(8):          # FC chunks 12..15
                    fc_half(12 + g // 2, g % 2)
                    if g % 2 == 1:
                        fin_half(*fins_b[g // 2])
                ar_batch(1)
                for mi in range(8, 16):
                    for hf in range(2):
                        fin_half(mi, hf)

    nc.compile()
    return nc


def _bf(x):
    return np.ascontiguousarray(
        np.asarray(x, np.float32).astype(ml_dtypes.bfloat16))


def _q8(x, scale):
    y = np.asarray(x, np.float32) * scale
    y = np.clip(y, -240.0, 240.0)
    return np.ascontiguousarray(y.astype(ml_dtypes.float8_e4m3))


def prep_inputs(target, encoder_outputs, enc_h0, enc_c0, emb, W_x, W_h,
                b_lstm, Wa, W_fc, b_fc, n_cores=NC):
    """Host-side layout prep + per-core sharding."""
    tgt = np.asarray(target).astype(np.int32)
    tidx = np.ascontiguousarray(tgt[:, :T].T.reshape(TB, 1))  # t-major rows
    enc = np.asarray(encoder_outputs, np.float32)
    # permute gate columns [i,f,g,o] -> [g,i,f,o]
    gperm = np.r_[2 * U:3 * U, 0:U, U:2 * U, 3 * U:4 * U]
    Wxp = np.asarray(W_x, np.float32)[:, gperm]
    # wx fp8 packed [128, (mi, j, m)]
    wx_pack = Wxp.reshape(2, 128, 16, 128).transpose(1, 2, 0, 3) \
        .reshape(128, 16 * 2 * 128)
    # W_h fp8 packed [128, (mi, kk, j, m)]
    Whp = np.asarray(W_h, np.float32)[:, gperm]          # [512, 2048]
    wh = Whp.reshape(4, 128, 16, 128)                    # [kt, p, mi, m]
    wh = wh.reshape(2, 2, 128, 16, 128)                  # [kk, j, p, mi, m]
    wh = wh.transpose(2, 3, 0, 1, 4).reshape(128, 16 * 2 * 2 * 128)
    # enc packed: enc_sb[0:64, j*U:] = enc[2j], [64:128] = enc[2j+1]
    enc_pack = np.empty((128, 16 * U), np.float32)
    for j in range(16):
        enc_pack[0:64, j * U:(j + 1) * U] = enc[2 * j]
        enc_pack[64:128, j * U:(j + 1) * U] = enc[2 * j + 1]
    # enct packed [128, (k, b, l)]
    enct = enc.transpose(2, 0, 1).reshape(U, BL)         # [U, (b,l)]
    enct_pack = enct.reshape(4, 128, BL).transpose(1, 0, 2).reshape(128, 4 * BL)
    # wa packed [128, (k, m)] with Wa/2 (absorbs H=2h in scores)
    wa = np.asarray(Wa, np.float32) * 0.5
    wa_pack = wa.reshape(4, 128, U).transpose(1, 0, 2).reshape(128, 4 * U)
    b_lstm = np.asarray(b_lstm, np.float32)
    lstm_bias = bool(np.any(b_lstm))
    common = {
        "tidx": tidx,
        "emb_bf": _bf(emb),
        "wx_q": _q8(wx_pack, SWX),
        "wh_q": _q8(wh, SWH),
        "enc_bf": _bf(enc_pack),
        "enct_bf": _bf(enct_pack),
        "wa_bf": _bf(wa_pack),
        "h0": np.ascontiguousarray(np.asarray(enc_h0, np.float32)),
        "c0": np.ascontiguousarray(np.asarray(enc_c0, np.float32)),
    }
    if lstm_bias:
        # lands in the zx PSUM, which carries scale SWX*SX
        common["wxb_bf"] = _bf(b_lstm[gperm].reshape(1, G4U) * SWX * SX)
    wfc = np.asarray(W_fc, np.float32)
    bfc = np.asarray(b_fc, np.float32)
    fc_bias = bool(np.any(bfc))
    in_maps = []
    for c in range(n_cores):
        m = dict(common)
        wshard = wfc[:, c * VS:(c + 1) * VS]             # [1024, VS]
        wsh = wshard.reshape(4, 2, 128, VS)              # [j, kk, p, v]
        wsh = wsh.transpose(2, 0, 3, 1).reshape(128, 4 * VS * 2)
        m["wfc_q"] = _q8(wsh, SW)
        if fc_bias:
            m["bfc_bf"] = _bf(bfc[c * VS:(c + 1) * VS].reshape(1, VS) * SP)
        in_maps.append(m)
    return in_maps, fc_bias, lstm_bias


def kernel(**inputs):
    in_maps, fc_bias, lstm_bias = prep_inputs(**inputs, n_cores=NC)
    key = ("nc", fc_bias, lstm_bias)
    if key not in _CACHE:
        _CACHE[key] = build(NC, fc_bias=fc_bias, lstm_bias=lstm_bias)
        _CACHE["nc"] = _CACHE[key]
    nc = _CACHE[key]
    res = run_bass_kernel_spmd(nc, in_maps, list(range(NC)))
    shards = [np.asarray(res.results[c]["out"]).astype(np.float32)
              for c in range(NC)]
    return np.concatenate(shards, axis=-1)


# revision 47
# speedup vs baseline: 1.3707x; 1.0109x over previous
"""Trainium2 Bass kernel for nn_Decoder_43336220016932.

Luong-attention LSTM decoder with teacher forcing:
  out[b,t,:] = log_softmax(tanh([ctx_t, h_t] @ W_fc + b_fc))

v5 strategy (8 NeuronCores):
  - Vocab-sharded tensor parallel: core i owns W_fc[:, i*4000:(i+1)*4000]
    in SBUF as fp8 (x256), k-pair interleaved for contiguous DoubleRow
    streaming. The serial LSTM recurrence is replicated on all cores;
    attention (d1) / FC / finalize are pipelined into its 63 steps.
  - Recurrence h@W_h in fp8 DoubleRow (W_h x1024 stationary, H8 = 16*h
    moving): 32 LDW+MM pairs/step. x@W_x is precomputed blockwise in fp8
    DoubleRow and staged in SBUF (fp8 x2048); added into gate PSUM by DVE.
  - All sigmoids as tanh with affine factors folded (c kept as 2c); one
    ACT table set. State is [U->partitions, B->free]; fp8 H8 feeds the
    recurrence, the attention scores AND the FC.
  - Attention: ep = enc@(Wa/2) in fp8 DoubleRow, stored fp8 (x1/8)
    interleaved; scores = H8 . ep8 in DoubleRow give RAW-scale scores.
  - logits tanh-bounded: sumexp = exp(x-1), no max pass; THREE AllReduce
    batches (chunks 0-7 / 8-13 / 14-15) so finalize starts early.
  - Finalize (logits - logZ) reloads logits from DRAM, alternates
    Vector/Scalar, writes bf16 output via gpsimd+sync DMA queues.
"""
from collections import defaultdict
from contextlib import ExitStack

import numpy as np
import ml_dtypes

import concourse.bass as bass
import concourse.tile as tile
from concourse import bacc, mybir
from concourse.bass_utils import run_bass_kernel_spmd
from concourse.masks import make_identity

B, S, L, U, E, V = 32, 64, 64, 512, 256, 32000
T = S - 1                  # 63 decode steps
NC = 8                     # cores
VS = V // NC               # 4000 vocab shard per core
TB = T * B                 # 2016 (t, b) rows, t-major
G4U = 4 * U                # 2048
BL = B * L                 # 2048
SW = 256.0                 # fp8 scale on W_fc
SA = 16.0                  # fp8 scale on ctx / h
SP = SW * SA               # product scale on logits in PSUM
SWH = 1024.0               # fp8 scale on W_h
SZ = SWH * SA              # scale of z in PSUM
SZX = 2048.0               # fp8 scale on staged zx
SX = 128.0                 # fp8 scale on gathered x
SWX = 1024.0               # fp8 scale on W_x
SWA = 2048.0               # fp8 scale on Wa/2
SE = 32.0                  # fp8 scale on enc (for ep matmul)
SEP = 0.125                # fp8 scale on stored ep (so scores are raw)
FILLER = 4                 # idle-bridging LDWEIGHTS per step
AF = mybir.ActivationFunctionType
ALU = mybir.AluOpType
AX = mybir.AxisListType
PM = mybir.MatmulPerfMode
F32 = mybir.dt.float32
BF16 = mybir.dt.bfloat16
FP8 = mybir.dt.float8e4
I32 = mybir.dt.int32
HALF = VS // 2             # 2000
QTR = VS // 4              # 1000
AR_RANGES = [(0, 8), (8, 14), (14, 16)]

_CACHE = {}


def build(n_cores=NC, fc_bias=False, lstm_bias=False):
    """Build the SPMD Bass program (same program on every core)."""
    nc = bacc.Bacc("TRN2", target_bir_lowering=False, debug=False,
                   num_devices=n_cores)

    # ---- external I/O ----
    tidx = nc.dram_tensor("tidx", [TB, 1], I32, kind="ExternalInput").ap()
    emb_bf = nc.dram_tensor("emb_bf", [V, E], BF16, kind="ExternalInput").ap()
    wx_q = nc.dram_tensor("wx_q", [128, 16 * 2 * 128], FP8,
                          kind="ExternalInput").ap()
    wh_q = nc.dram_tensor("wh_q", [128, 16 * 2 * 2 * 128], FP8,
                          kind="ExternalInput").ap()
    enc_bf = nc.dram_tensor("enc_bf", [128, 16 * U], BF16,
                            kind="ExternalInput").ap()
    # encT fp8 interleaved [p, (kk, c=BL, jj)] x SE
    enct_q = nc.dram_tensor("enct_q", [128, 2 * BL * 2], FP8,
                            kind="ExternalInput").ap()
    # Wa/2 fp8 [p, (mu, kk, jj, m)] x SWA
    wa_q = nc.dram_tensor("wa_q", [128, 4 * 2 * 2 * 128], FP8,
                          kind="ExternalInput").ap()
    h0 = nc.dram_tensor("h0", [B, U], F32, kind="ExternalInput").ap()
    c0 = nc.dram_tensor("c0", [B, U], F32, kind="ExternalInput").ap()
    wfc_q = nc.dram_tensor("wfc_q", [128, 4 * VS * 2], FP8,
                           kind="ExternalInput").ap()
    if lstm_bias:
        wxb_in = nc.dram_tensor("wxb_bf", [1, G4U], BF16,
                                kind="ExternalInput").ap()
    if fc_bias:
        bfc_in = nc.dram_tensor("bfc_bf", [1, VS], BF16,
                                kind="ExternalInput").ap()
    out = nc.dram_tensor("out", [B, T, VS], BF16, kind="ExternalOutput").ap()

    with tile.TileContext(nc) as tc, ExitStack() as perm:
        # ---------------- permanent pools ----------------
        konst = perm.enter_context(tc.tile_pool(name="konst", bufs=1))
        wpool = perm.enter_context(tc.tile_pool(name="wpool", bufs=1))
        hpool = perm.enter_context(tc.tile_pool(name="hpool", bufs=1))
        dram = perm.enter_context(tc.tile_pool(name="dram", bufs=1, space="DRAM"))
        stats = perm.enter_context(tc.tile_pool(name="stats", bufs=1))

        idt = konst.tile([128, 128], BF16)
        make_identity(nc, idt[:])
        negone = konst.tile([128, 1], F32)
        nc.vector.memset(negone[:], -1.0)
        idtf = konst.tile([128, 128], F32)
        make_identity(nc, idtf[:])
        ones_bf = konst.tile([1, 512], BF16)
        nc.vector.memset(ones_bf[:], 1.0)

        wfc_sb = wpool.tile([128, 4 * VS * 2], FP8)
        wfc_k = wfc_sb[:].rearrange("p (j v k) -> p j k v", j=4, k=2)
        wh_sb = wpool.tile([128, 16 * 2 * 2 * 128], FP8)
        wh_k = wh_sb[:].rearrange("p (mi kk j m) -> p mi kk j m",
                                  mi=16, kk=2, j=2)
        if fc_bias:
            bfc_row = wpool.tile([1, VS], BF16)

        # H8: fp8 x16 h (recurrence + scores + FC). col = k*2048+slot*32+b
        H8 = hpool.tile([128, 4 * 64 * B], FP8)
        H8k = H8[:].rearrange("p (k s b) -> p k s b", k=4, s=64)
        H8r = H8[:].rearrange("p (k c) -> p k c", k=4)
        # G_ctx: ctx.T fp8 x16, col = k*2016 + t*32 + b
        Gc = hpool.tile([128, 4 * TB], FP8)
        Gck = Gc[:].rearrange("p (k t b) -> p k t b", k=4, t=T)
        Gcr = Gc[:].rearrange("p (k r) -> p k r", k=4)
        cT = hpool.tile([128, 128], F32)     # (2c).T state, col = k*32+b

        # Zx staged in SBUF: [p, (t, mi, b)] fp8, values x SZX
        zxt_sb = hpool.tile([128, T * 512], FP8)
        zxt_t = zxt_sb[:].rearrange("p (t c) -> p t c", t=T)
        zxt_blk = zxt_sb[:].rearrange("p (t m b) -> p m t b", t=T, m=16)

        # ep fp8 [p, (kk, b, l, jj)] x SEP
        epT_sb = hpool.tile([128, 2 * BL * 2], FP8)
        e8v = epT_sb[:].rearrange("p (kk b l jj) -> p kk jj b l",
                                  kk=2, b=B, jj=2)
        e8w = epT_sb[:].rearrange("p (kk c jj) -> p kk jj c", kk=2, jj=2)
        enc_sb = hpool.tile([128, 16 * U], BF16)    # 2 b per 64-row group

        # per-row ((t,b) grouped [128 x 16]) log-softmax stats.
        lsum_sb = stats.tile([128, 16], F32)   # local sum exp(x - 1)
        sg_sb = stats.tile([128, 16], F32)     # global sum
        logz_sb = stats.tile([128, 16], F32)   # ln(global sum)
        nlz_sb = stats.tile([128, 16], F32)    # -(1 + ln(global sum))
        nc.vector.memset(lsum_sb[:], 1.0)

        # DRAM scratch
        logits_d = dram.tile([16, 128, VS], BF16)    # tanh'd logits
        ccs_in = [dram.tile([128, cb - ca], F32, name=f"cci{i}")
                  for i, (ca, cb) in enumerate(AR_RANGES)]
        ccs_out = [dram.tile([128, cb - ca], F32, name=f"cco{i}")
                   for i, (ca, cb) in enumerate(AR_RANGES)]

        d1w = perm.enter_context(tc.tile_pool(name="d1w", bufs=3))
        d1s = perm.enter_context(tc.tile_pool(name="d1s", bufs=3))
        sst = perm.enter_context(tc.tile_pool(name="sst", bufs=6))

        lg_tiles = {}
        ac_tiles = {}

        # =========== schedulable work units (emitted into R) ===========
        pools = {}

        def fc_q(mi, qi):
            """FC chunk mi, vocab quarter qi: fp8 DoubleRow + tanh + exp."""
            r0 = mi * 128
            rows = min(128, TB - r0)
            if qi == 0:
                lg_tiles[mi] = pools["lgp"].tile([128, VS], BF16, tag="lg",
                                                 name="lg")
                ac_tiles[mi] = []
            lg = lg_tiles[mi]
            base = qi * QTR
            fcp = pools["fps"].tile([128, QTR], F32, tag="fc")
            for j in range(4):
                lhs = Gcr[:, 2 * j: 2 * j + 2, r0:r0 + rows] if j < 2 \
                    else H8r[:, 2 * (j - 2): 2 * (j - 2) + 2,
                             B + r0: B + r0 + rows]
                for off, w in [(0, 512), (512, 488)]:
                    nc.tensor.matmul(
                        fcp[:rows, off:off + w], lhs,
                        wfc_k[:, j, :, base + off: base + off + w],
                        start=(j == 0), stop=(j == 3) and not fc_bias,
                        perf_mode=PM.DoubleRow)
            if fc_bias:
                for off, w in [(0, 512), (512, 488)]:
                    nc.tensor.matmul(
                        fcp[:rows, off:off + w],
                        ones_bf[:1, :rows],
                        bfc_row[:1, base + off: base + off + w],
                        start=False, stop=True,
                        skip_group_check=True)
            qs = slice(base, base + QTR)
            nc.scalar.activation(lg[:rows, qs], fcp[:rows, :], AF.Tanh,
                                 scale=1.0 / SP)
            sc_ = pools["scr"].tile([128, QTR], BF16, tag="sc")
            acx = sst.tile([128, 1], F32, tag="ac")
            nc.scalar.activation(sc_[:rows, :], lg[:rows, qs], AF.Exp,
                                 bias=negone[:rows, :],
                                 accum_out=acx[:rows, :])
            nc.sync.dma_start(logits_d[mi, :rows, qs], lg[:rows, qs])
            ac_tiles[mi].append(acx)
            if qi == 3:
                a = ac_tiles[mi]
                s0_ = sst.tile([128, 1], F32, tag="hs")
                s1_ = sst.tile([128, 1], F32, tag="hs2")
                nc.vector.tensor_add(s0_[:rows, :], a[0][:rows, :],
                                     a[1][:rows, :])
                nc.vector.tensor_add(s1_[:rows, :], a[2][:rows, :],
                                     a[3][:rows, :])
                nc.vector.tensor_add(lsum_sb[:rows, mi:mi + 1],
                                     s0_[:rows, :], s1_[:rows, :])

        def ar_batch(bi):
            """AllReduce batch bi's sumexp; nlz = -(1 + ln S)."""
            ca, cb = AR_RANGES[bi]
            nc.sync.dma_start(ccs_in[bi][:], lsum_sb[:, ca:cb])
            nc.gpsimd.collective_compute(
                "AllReduce", ALU.add,
                replica_groups=[list(range(n_cores))],
                ins=[ccs_in[bi][:].opt()], outs=[ccs_out[bi][:].opt()])
            nc.gpsimd.dma_start(sg_sb[:, ca:cb], ccs_out[bi][:])
            nc.scalar.activation(logz_sb[:, ca:cb], sg_sb[:, ca:cb], AF.Ln)
            nc.vector.tensor_scalar(nlz_sb[:, ca:cb], logz_sb[:, ca:cb],
                                    -1.0, -1.0, op0=ALU.mult, op1=ALU.add)

        def fin_half(mi, half):
            """out = logits - (1 + lnS), reloading logits from DRAM."""
            r0 = mi * 128
            rows = min(128, TB - r0)
            hs = slice(half * HALF, (half + 1) * HALF)
            ob = pools["fin"].tile([128, HALF], BF16, tag="ob")
            nc.sync.dma_start(ob[:rows, :], logits_d[mi, :rows, hs])
            if (2 * mi + half) % 2 == 0:
                nc.vector.tensor_scalar(
                    ob[:rows, :], ob[:rows, :],
                    nlz_sb[:rows, mi:mi + 1], None, op0=ALU.add)
            else:
                nc.scalar.activation(ob[:rows, :], ob[:rows, :],
                                     AF.Identity,
                                     bias=nlz_sb[:rows, mi:mi + 1])
            t0 = mi * 4
            eng = nc.gpsimd if (2 * mi + half) % 2 == 0 else nc.sync
            for tl in range(rows // B):
                eng.dma_start(out[:, t0 + tl, hs],
                              ob[tl * B:(tl + 1) * B, :])

        def d1_sub(s0, nt, u):
            """Attention sub-unit: j-pairs 2u, 2u+1 of a step block.

            Block covers h slots s0..s0+nt-1 -> out-t s0-1..s0+nt-2.
            scores (fp8 DR, raw scale) -> softmax -> attn.T -> ctx.T ->
            Gc (fp8 x16).
            """
            for j in (2 * u, 2 * u + 1):
                # both batch-halves at partition base 0 (DR dst rule),
                # separated by 64-column blocks
                scp = pools["pps"].tile([128, 512], F32, tag="zx")
                for hf in range(2):
                    b = 2 * j + hf
                    cs = slice(64 * hf, 64 * hf + 64)
                    for kk in range(2):
                        nc.tensor.matmul(
                            scp[:nt, cs],
                            H8k[:, 2 * kk:2 * kk + 2, s0:s0 + nt, b],
                            e8v[:, kk, :, b, :],
                            start=(kk == 0), stop=(kk == 1),
                            perf_mode=PM.DoubleRow)
                att_f = d1w.tile([128, 128], F32, tag="af")
                attb = d1w.tile([128, 128], BF16, tag="ab")
                for hf in range(2):
                    cs = slice(64 * hf, 64 * hf + 64)
                    nmx = d1s.tile([128, 1], F32, tag="nm")
                    nc.vector.tensor_reduce(nmx[:nt, :], scp[:nt, cs],
                                            axis=AX.X, op=ALU.max,
                                            negate=True)
                    ssum = d1s.tile([128, 1], F32, tag="ss")
                    nc.scalar.activation(att_f[:nt, cs], scp[:nt, cs],
                                         AF.Exp, bias=nmx[:nt, :],
                                         accum_out=ssum[:nt, :])
                    rcp = d1s.tile([128, 1], F32, tag="rc")
                    nc.vector.reciprocal(rcp[:nt, :], ssum[:nt, :])
                    nc.vector.tensor_scalar_mul(attb[:nt, cs],
                                                att_f[:nt, cs],
                                                rcp[:nt, :])
                atT = d1w.tile([128, 16], BF16, tag="atT")
                for hf in range(2):
                    po = 64 * hf
                    cs = slice(64 * hf, 64 * hf + 64)
                    tpp = pools["tpsA"].tile([128, 128], BF16, tag="tpb")
                    nc.tensor.transpose(tpp[po:po + L, :nt],
                                        attb[:nt, cs],
                                        idt[:nt, :nt])
                    nc.vector.tensor_copy(atT[po:po + L, :nt],
                                          tpp[po:po + L, :nt])
                for hf in range(2):
                    b = 2 * j + hf
                    po = 64 * hf
                    ctp = pools["pps"].tile([128, 512], F32, tag="zx")
                    for mu in range(4):
                        nc.tensor.matmul(
                            ctp[:, mu * nt:(mu + 1) * nt],
                            enc_sb[po:po + L,
                                   j * U + mu * 128: j * U + (mu + 1) * 128],
                            atT[po:po + L, :nt],
                            start=True, stop=True)
                    nc.vector.tensor_scalar_mul(
                        Gck[:, :, s0 - 1: s0 - 1 + nt, b],
                        ctp[:, :4 * nt].rearrange("p (k tt) -> p k tt", k=4),
                        SA)

        # ================================================================
        with ExitStack() as pscope:
            psb = pscope.enter_context(tc.tile_pool(name="p_sbuf", bufs=2))
            pps = pscope.enter_context(
                tc.tile_pool(name="p_psum", bufs=2, space="PSUM"))
            pools["pps"] = pps
            rzp = pscope.enter_context(
                tc.tile_pool(name="r_zps", bufs=1, space="PSUM"))
            pools["fps"] = pscope.enter_context(
                tc.tile_pool(name="fc_psum", bufs=2, space="PSUM"))
            pools["tpsA"] = pscope.enter_context(
                tc.tile_pool(name="tpa_ps", bufs=1, space="PSUM"))
            rga = pscope.enter_context(tc.tile_pool(name="r_gate", bufs=2))

            # pools released mid-R to free SBUF
            zscope = ExitStack()
            zwp = zscope.enter_context(tc.tile_pool(name="zwp", bufs=1))
            # x.T fp8 interleaved: col = row*2 + j  (j = E-chunk), x SX
            xt_sb = zwp.tile([128, 2 * TB], FP8)
            xt_j = xt_sb[:].rearrange("p (r j) -> p j r", j=2)
            wx_sb = zwp.tile([128, 16 * 2 * 128], FP8)
            wx_k = wx_sb[:].rearrange("p (mi j m) -> p mi j m", mi=16, j=2)
            if lstm_bias:
                wxb_sb = zwp.tile([1, G4U], BF16)
            escope = ExitStack()
            ewp = escope.enter_context(tc.tile_pool(name="ewp", bufs=1))
            enct_sb = ewp.tile([128, 2 * BL * 2], FP8)
            e8in = enct_sb[:].rearrange("p (kk c jj) -> p kk jj c",
                                        kk=2, jj=2)
            wa_sb = ewp.tile([128, 4 * 2 * 2 * 128], FP8)
            wa_k = wa_sb[:].rearrange("p (mu kk jj m) -> p mu kk jj m",
                                      mu=4, kk=2, jj=2)

            def emit_zx_unit(nb, mi):
                """One (t-block, m-chunk) unit of Zx.T = W_x.T@X.T (+b)."""
                t0 = nb * 16
                tn = min(16, T - t0)
                ncols = tn * B
                zps = pps.tile([128, 512], F32, tag="zx")
                nc.tensor.matmul(
                    zps[:, :ncols],
                    wx_k[:, mi],
                    xt_j[:, :, t0 * B: t0 * B + ncols],
                    start=True, stop=not lstm_bias,
                    perf_mode=PM.DoubleRow)
                if lstm_bias:
                    nc.tensor.matmul(zps[:, :ncols],
                                     wxb_sb[:1, mi * 128:(mi + 1) * 128],
                                     ones_bf[:1, :ncols],
                                     start=False, stop=True,
                                     skip_group_check=True)
                nc.vector.tensor_scalar_mul(
                    zxt_blk[:, mi, t0:t0 + tn, :],
                    zps[:, :ncols].rearrange("p (t b) -> p t b", b=B),
                    SZX / (SWX * SX))

            def emit_ep_unit(un):
                """ep8 = (enc @ Wa/2) x SEP, unit (mu, nb), fp8 DR."""
                mu, nb = un // 4, un % 4
                kk, jj = mu // 2, mu % 2
                eps_ = pps.tile([128, 512], F32, tag="zx")
                for k2 in range(2):
                    nc.tensor.matmul(
                        eps_[:, :],
                        wa_k[:, mu, k2],
                        e8in[:, k2, :, nb * 512:(nb + 1) * 512],
                        start=(k2 == 0), stop=(k2 == 1),
                        perf_mode=PM.DoubleRow)
                nc.vector.tensor_scalar_mul(
                    e8w[:, kk, jj, nb * 512:(nb + 1) * 512],
                    eps_[:], SEP / (SWA * SE))

            # ============ phase P: minimal preamble ============
            def emit_gather(i):
                r0 = i * 128
                rows = min(128, TB - r0)
                ix = psb.tile([128, 1], I32, tag="ix")
                nc.sync.dma_start(ix[:rows, :], tidx[r0:r0 + rows, :])
                xg = psb.tile([128, E], BF16, tag="xg")
                nc.gpsimd.indirect_dma_start(
                    out=xg[:rows, :], out_offset=None,
                    in_=emb_bf[:],
                    in_offset=bass.IndirectOffsetOnAxis(
                        ap=ix[:rows, :1], axis=0),
                )
                for cc in range(2):
                    tp = pools["tpsA"].tile([128, 128], BF16, tag="tpb")
                    nc.tensor.transpose(
                        tp[:, :rows],
                        xg[:rows, cc * 128:(cc + 1) * 128],
                        idt[:rows, :rows])
                    nc.vector.tensor_scalar_mul(
                        xt_j[:, cc, r0: r0 + rows],
                        tp[:, :rows], SX)

            # big packed weight loads first (spread across DGE queues)
            nc.sync.dma_start(wx_sb[:], wx_q[:])
            nc.scalar.dma_start(wh_sb[:], wh_q[:])
            if lstm_bias:
                nc.sync.dma_start(wxb_sb[:], wxb_in[:])

            for i in range(4):
                emit_gather(i)

            # h0/c0 init: H8 = SA*h, cT = 2c
            hc_sb = psb.tile([B, U], F32, tag="hc")
            nc.sync.dma_start(hc_sb[:, :], h0[:, :])
            cc_sb = psb.tile([B, U], F32, tag="hc2")
            nc.sync.dma_start(cc_sb[:, :], c0[:, :])
            for k in range(4):
                tp = pps.tile([128, 512], F32, tag="zx")
                nc.tensor.transpose(
                    tp[:, :B], hc_sb[:B, k * 128:(k + 1) * 128],
                    idtf[:B, :B])
                nc.vector.tensor_scalar_mul(H8k[:, k, 0, :],
                                            tp[:, :B], SA)
                tp2 = pps.tile([128, 512], F32, tag="zx")
                nc.tensor.transpose(
                    tp2[:, :B], cc_sb[:B, k * 128:(k + 1) * 128],
                    idtf[:B, :B])
                nc.vector.tensor_scalar_mul(
                    cT[:, k * B:(k + 1) * B], tp2[:, :B], 2.0)

            for mi in range(16):
                emit_zx_unit(0, mi)

            # remaining loads (all overlap the early recurrence)
            nc.scalar.dma_start(wfc_sb[:], wfc_q[:])
            if fc_bias:
                nc.sync.dma_start(bfc_row[:, :], bfc_in[:, :])
            nc.scalar.dma_start(enct_sb[:], enct_q[:])
            nc.sync.dma_start(wa_sb[:], wa_q[:])
            nc.sync.dma_start(enc_sb[:], enc_bf[:])

            # ============ phase R: the master pipeline ============
            if True:
                # ---- interleave schedule: step -> work units ----
                sched = defaultdict(list)
                for t in range(0, 6):       # gathers 4..15
                    sched[t] += [lambda i=4 + 2 * t_ + k_: emit_gather(i)
                                 for t_, k_ in [(t, 0), (t, 1)]]
                for t in range(2, 10):      # Zx block 1 + ep 0..7
                    g = t - 2
                    sched[t] += [lambda m=2 * g: emit_zx_unit(1, m),
                                 lambda m=2 * g + 1: emit_zx_unit(1, m),
                                 lambda un=g: emit_ep_unit(un)]
                for t in range(10, 18):     # Zx block 2 + ep 8..15
                    g = t - 10
                    sched[t] += [lambda m=2 * g: emit_zx_unit(2, m),
                                 lambda m=2 * g + 1: emit_zx_unit(2, m),
                                 lambda un=8 + g: emit_ep_unit(un)]
                sched[18] += [escope.close]
                for t in range(16, 20):     # D1 block A (t 0..15)
                    g = t - 16
                    sched[t] += [lambda u=2 * g: d1_sub(1, 16, u),
                                 lambda u=2 * g + 1: d1_sub(1, 16, u)]

                def open_fc_pools():
                    pools["lgp"] = zscope.enter_context(
                        tc.tile_pool(name="lgp", bufs=2))
                    pools["scr"] = zscope.enter_context(
                        tc.tile_pool(name="scr", bufs=2))
                    pools["fin"] = zscope.enter_context(
                        tc.tile_pool(name="fin", bufs=3))
                sched[19] += [open_fc_pools]
                for t in range(20, 28):     # FC chunks 0..3 (16 q)
                    g = t - 20
                    sched[t] += [lambda mi=g // 2, qi=2 * (g % 2):
                                 fc_q(mi, qi),
                                 lambda mi=g // 2, qi=2 * (g % 2) + 1:
                                 fc_q(mi, qi)]
                for t in range(28, 36):     # Zx block 3
                    g = t - 28
                    sched[t] += [lambda m=2 * g: emit_zx_unit(3, m),
                                 lambda m=2 * g + 1: emit_zx_unit(3, m)]
                for t in range(32, 36):     # D1 block B (t 16..31)
                    g = t - 32
                    sched[t] += [lambda u=2 * g: d1_sub(17, 16, u),
                                 lambda u=2 * g + 1: d1_sub(17, 16, u)]
                for t in range(36, 44):     # FC chunks 4..7
                    g = t - 36
                    sched[t] += [lambda mi=4 + g // 2, qi=2 * (g % 2):
                                 fc_q(mi, qi),
                                 lambda mi=4 + g // 2, qi=2 * (g % 2) + 1:
                                 fc_q(mi, qi)]
                sched[44] += [lambda: ar_batch(0)]
                for t in range(48, 52):     # D1 block C (t 32..47)
                    g = t - 48
                    sched[t] += [lambda u=2 * g: d1_sub(33, 16, u),
                                 lambda u=2 * g + 1: d1_sub(33, 16, u)]
                for t in range(47, 62):     # fins chunks 0..7 (batch 0)
                    g = t - 47
                    if g < 16:
                        sched[t] += [lambda mi=g // 2, hf=g % 2:
                                     fin_half(mi, hf)]
                sched[62] += [lambda: fin_half(7, 1)]
                for t in range(52, 56):     # FC chunks 8..9
                    g = t - 52
                    sched[t] += [lambda mi=8 + g // 2, qi=2 * (g % 2):
                                 fc_q(mi, qi),
                                 lambda mi=8 + g // 2, qi=2 * (g % 2) + 1:
                                 fc_q(mi, qi)]
                for t in range(56, 60):     # D1 block D (t 48..55)
                    g = t - 56
                    sched[t] += [lambda u=2 * g: d1_sub(49, 8, u),
                                 lambda u=2 * g + 1: d1_sub(49, 8, u)]
                for t in range(60, 63):     # FC chunk 10 + 11.0
                    g = t - 60
                    sched[t] += [lambda mi=10 + g // 2, qi=2 * (g % 2):
                                 fc_q(mi, qi),
                                 lambda mi=10 + g // 2, qi=2 * (g % 2) + 1:
                                 fc_q(mi, qi)]

                c_prev = cT
                for t in range(T):
                    for _ in range(FILLER):
                        nc.tensor.ldweights(idt[:])
                    zps = rzp.tile([128, 512], F32, tag="zt")
                    # gate order [g, i, f, o] (host-permuted);
                    # sigmoid(z) = (1+tanh(z/2))/2, folded.
                    gate = {}
                    for gi in range(4):
                        for m2 in range(4):
                            mi = gi * 4 + m2
                            for kk in range(2):
                                nc.tensor.matmul(
                                    zps[:, mi * B:(mi + 1) * B],
                                    wh_k[:, mi, kk],
                                    H8k[:, 2 * kk:2 * kk + 2, t, :],
                                    start=(kk == 0), stop=(kk == 1),
                                    perf_mode=PM.DoubleRow)
                        sl = slice(gi * 128, (gi + 1) * 128)
                        # zq = z + zx*(SZ/SZX) (staged fp8), into SBUF
                        zq = rga.tile([128, 128], F32, tag=f"q{gi}",
                                      name=f"q{gi}")
                        nc.vector.scalar_tensor_tensor(
                            zq[:], zxt_t[:, t, sl], SZ / SZX,
                            zps[:, sl], op0=ALU.mult, op1=ALU.add)
                        gt = rga.tile([128, 128], F32, tag=f"g{gi}",
                                      name=f"g{gi}")
                        nc.scalar.activation(
                            gt[:], zq[:], AF.Tanh,
                            scale=(1.0 if gi == 0 else 0.5) / SZ)
                        gate[gi] = gt
                        if gi == 1:
                            # Bv = (1+ti)*tg  (= 2*i*g)
                            ig = rga.tile([128, 128], F32, tag="ig")
                            nc.vector.scalar_tensor_tensor(
                                ig[:], gate[1][:], 1.0, gate[0][:],
                                op0=ALU.add, op1=ALU.mult)
                        elif gi == 2:
                            # A = (1+tf)*st ; st_new = A/2 + Bv
                            fc_ = rga.tile([128, 128], F32, tag="fc")
                            nc.vector.scalar_tensor_tensor(
                                fc_[:], gate[2][:], 1.0, c_prev[:],
                                op0=ALU.add, op1=ALU.mult)
                            c_new = rga.tile([128, 128], F32, tag="cn")
                            nc.vector.scalar_tensor_tensor(
                                c_new[:], fc_[:], 0.5, ig[:],
                                op0=ALU.mult, op1=ALU.add)
                            tc_ = rga.tile([128, 128], F32, tag="tc")
                            nc.scalar.activation(tc_[:], c_new[:],
                                                 AF.Tanh, scale=0.5)
                    # H8(t+1) = SA*h = ((SA/2)(1+to)) * tanh(c)
                    g3p = rga.tile([128, 128], F32, tag="g3p")
                    nc.vector.tensor_scalar(g3p[:], gate[3][:],
                                            SA / 2.0, SA / 2.0,
                                            op0=ALU.mult, op1=ALU.add)
                    nc.vector.tensor_tensor(
                        H8k[:, :, t + 1, :],
                        g3p[:].rearrange("p (k b) -> p k b", k=4),
                        tc_[:].rearrange("p (k b) -> p k b", k=4),
                        op=ALU.mult)
                    c_prev = c_new
                    for unit in sched[t]:
                        unit()

                # =================== tail ===================
                fc_q(11, 2)
                fc_q(11, 3)
                for g in range(4):          # FC chunks 12..13
                    fc_q(12 + g // 2, 2 * (g % 2))
                    fc_q(12 + g // 2, 2 * (g % 2) + 1)
                ar_batch(1)                 # chunks 8..13
                for u in range(8):          # D1 block E (t 56..62)
                    d1_sub(57, 7, u)
                    if u % 2 == 1:
                        fin_half(8 + u // 2, 0)
                        fin_half(8 + u // 2, 1)
                for g in range(4):          # FC chunks 14..15
                    fc_q(14 + g // 2, 2 * (g % 2))
                    fc_q(14 + g // 2, 2 * (g % 2) + 1)
                    if g >= 2:
                        fin_half(10 + g, 0)
                        fin_half(10 + g, 1)
                ar_batch(2)                 # chunks 14..15
                for mi in range(14, 16):
                    for hf in range(2):
                        fin_half(mi, hf)
                zscope.close()

    nc.compile()
    return nc


def _bf(x):
    return np.ascontiguousarray(
        np.asarray(x, np.float32).astype(ml_dtypes.bfloat16))


def _q8(x, scale):
    y = np.asarray(x, np.float32) * scale
    y = np.clip(y, -240.0, 240.0)
    return np.ascontiguousarray(y.astype(ml_dtypes.float8_e4m3))


def prep_inputs(target, encoder_outputs, enc_h0, enc_c0, emb, W_x, W_h,
                b_lstm, Wa, W_fc, b_fc, n_cores=NC):
    """Host-side layout prep + per-core sharding."""
    tgt = np.asarray(target).astype(np.int32)
    tidx = np.ascontiguousarray(tgt[:, :T].T.reshape(TB, 1))  # t-major rows
    enc = np.asarray(encoder_outputs, np.float32)
    # permute gate columns [i,f,g,o] -> [g,i,f,o]
    gperm = np.r_[2 * U:3 * U, 0:U, U:2 * U, 3 * U:4 * U]
    Wxp = np.asarray(W_x, np.float32)[:, gperm]
    # wx fp8 packed [128, (mi, j, m)]
    wx_pack = Wxp.reshape(2, 128, 16, 128).transpose(1, 2, 0, 3) \
        .reshape(128, 16 * 2 * 128)
    # W_h fp8 packed [128, (mi, kk, j, m)]
    Whp = np.asarray(W_h, np.float32)[:, gperm]          # [512, 2048]
    wh = Whp.reshape(4, 128, 16, 128)                    # [kt, p, mi, m]
    wh = wh.reshape(2, 2, 128, 16, 128)                  # [kk, j, p, mi, m]
    wh = wh.transpose(2, 3, 0, 1, 4).reshape(128, 16 * 2 * 2 * 128)
    # enc packed: enc_sb[0:64, j*U:] = enc[2j], [64:128] = enc[2j+1]
    enc_pack = np.empty((128, 16 * U), np.float32)
    for j in range(16):
        enc_pack[0:64, j * U:(j + 1) * U] = enc[2 * j]
        enc_pack[64:128, j * U:(j + 1) * U] = enc[2 * j + 1]
    # encT fp8 interleaved [p, (kk, c, jj)]: elem = enc.T[(2kk+jj)*128+p, c]
    enct = enc.transpose(2, 0, 1).reshape(U, BL)         # [U, (b,l)]
    e4 = enct.reshape(2, 2, 128, BL)                     # [kk, jj, p, c]
    enct_pack = e4.transpose(2, 0, 3, 1).reshape(128, 2 * BL * 2)
    # Wa/2 fp8 [p, (mu, kk, jj, m)]: row = (2kk+jj)*128+p, col = mu*128+m
    wa = np.asarray(Wa, np.float32) * 0.5
    wa4 = wa.reshape(2, 2, 128, 4, 128)                  # [kk, jj, p, mu, m]
    wa_pack = wa4.transpose(2, 3, 0, 1, 4).reshape(128, 4 * 2 * 2 * 128)
    b_lstm = np.asarray(b_lstm, np.float32)
    lstm_bias = bool(np.any(b_lstm))
    common = {
        "tidx": tidx,
        "emb_bf": _bf(emb),
        "wx_q": _q8(wx_pack, SWX),
        "wh_q": _q8(wh, SWH),
        "enc_bf": _bf(enc_pack),
        "enct_q": _q8(enct_pack, SE),
        "wa_q": _q8(wa_pack, SWA),
        "h0": np.ascontiguousarray(np.asarray(enc_h0, np.float32)),
        "c0": np.ascontiguousarray(np.asarray(enc_c0, np.float32)),
    }
    if lstm_bias:
        # lands in the zx PSUM, which carries scale SWX*SX
        common["wxb_bf"] = _bf(b_lstm[gperm].reshape(1, G4U) * SWX * SX)
    wfc = np.asarray(W_fc, np.float32)
    bfc = np.asarray(b_fc, np.float32)
    fc_bias = bool(np.any(bfc))
    in_maps = []
    for c in range(n_cores):
        m = dict(common)
        wshard = wfc[:, c * VS:(c + 1) * VS]             # [1024, VS]
        wsh = wshard.reshape(4, 2, 128, VS)              # [j, kk, p, v]
        wsh = wsh.transpose(2, 0, 3, 1).reshape(128, 4 * VS * 2)
        m["wfc_q"] = _q8(wsh, SW)
        if fc_bias:
            m["bfc_bf"] = _bf(bfc[c * VS:(c + 1) * VS].reshape(1, VS) * SP)
        in_maps.append(m)
    return in_maps, fc_bias, lstm_bias


def kernel(**inputs):
    in_maps, fc_bias, lstm_bias = prep_inputs(**inputs, n_cores=NC)
    key = ("nc", fc_bias, lstm_bias)
    if key not in _CACHE:
        _CACHE[key] = build(NC, fc_bias=fc_bias, lstm_bias=lstm_bias)
        _CACHE["nc"] = _CACHE[key]
    nc = _CACHE[key]
    res = run_bass_kernel_spmd(nc, in_maps, list(range(NC)))
    shards = [np.asarray(res.results[c]["out"]).astype(np.float32)
              for c in range(NC)]
    return np.concatenate(shards, axis=-1)


# revision 57
# speedup vs baseline: 1.3966x; 1.0189x over previous
"""Trainium2 Bass kernel for nn_Decoder_43336220016932.

Luong-attention LSTM decoder with teacher forcing:
  out[b,t,:] = log_softmax(tanh([ctx_t, h_t] @ W_fc + b_fc))

v5 strategy (8 NeuronCores):
  - Vocab-sharded tensor parallel: core i owns W_fc[:, i*4000:(i+1)*4000]
    in SBUF as fp8 (x256), k-pair interleaved for contiguous DoubleRow
    streaming. The serial LSTM recurrence is replicated on all cores;
    attention (d1) / FC / finalize are pipelined into its 63 steps.
  - Recurrence h@W_h in fp8 DoubleRow (W_h x1024 stationary, H8 = 16*h
    moving): 32 LDW+MM pairs/step. x@W_x is precomputed blockwise in fp8
    DoubleRow and staged in SBUF (fp8 x2048); added into gate PSUM by DVE.
  - All sigmoids as tanh with affine factors folded (c kept as 2c); one
    ACT table set. State is [U->partitions, B->free]; fp8 H8 feeds the
    recurrence, the attention scores AND the FC.
  - Attention: ep = enc@(Wa/2) in fp8 DoubleRow, stored fp8 (x1/8)
    interleaved; scores = H8 . ep8 in DoubleRow give RAW-scale scores.
  - logits tanh-bounded: sumexp = exp(x-1), no max pass; THREE AllReduce
    batches (chunks 0-7 / 8-13 / 14-15) so finalize starts early.
  - Finalize (logits - logZ) reloads logits from DRAM, alternates
    Vector/Scalar, writes bf16 output via gpsimd+sync DMA queues.
"""
from collections import defaultdict
from contextlib import ExitStack

import numpy as np
import ml_dtypes

import concourse.bass as bass
import concourse.tile as tile
from concourse import bacc, mybir
from concourse.bass_utils import run_bass_kernel_spmd
from concourse.masks import make_identity

B, S, L, U, E, V = 32, 64, 64, 512, 256, 32000
T = S - 1                  # 63 decode steps
NC = 8                     # cores
VS = V // NC               # 4000 vocab shard per core
TB = T * B                 # 2016 (t, b) rows, t-major
G4U = 4 * U                # 2048
BL = B * L                 # 2048
SW = 256.0                 # fp8 scale on W_fc
SA = 16.0                  # fp8 scale on ctx / h
SP = SW * SA               # product scale on logits in PSUM
SWH = 1024.0               # fp8 scale on W_h
SZ = SWH * SA              # scale of z in PSUM
SZX = 2048.0               # fp8 scale on staged zx
SX = 128.0                 # fp8 scale on gathered x
SWX = 1024.0               # fp8 scale on W_x
SWA = 2048.0               # fp8 scale on Wa/2
SE = 32.0                  # fp8 scale on enc (for ep matmul)
SEP = 0.125                # fp8 scale on stored ep (so scores are raw)
FILLER = 8                 # idle-bridging LDWEIGHTS per step
AF = mybir.ActivationFunctionType
ALU = mybir.AluOpType
AX = mybir.AxisListType
PM = mybir.MatmulPerfMode
F32 = mybir.dt.float32
BF16 = mybir.dt.bfloat16
FP8 = mybir.dt.float8e4
I32 = mybir.dt.int32
HALF = VS // 2             # 2000
QTR = VS // 4              # 1000
AR_RANGES = [(0, 8), (8, 14), (14, 16)]

_CACHE = {}


def build(n_cores=NC, fc_bias=False, lstm_bias=False):
    """Build the SPMD Bass program (same program on every core)."""
    nc = bacc.Bacc("TRN2", target_bir_lowering=False, debug=False,
                   num_devices=n_cores)

    # ---- external I/O ----
    tidx = nc.dram_tensor("tidx", [TB, 1], I32, kind="ExternalInput").ap()
    emb_bf = nc.dram_tensor("emb_bf", [V, E], BF16, kind="ExternalInput").ap()
    wx_q = nc.dram_tensor("wx_q", [128, 16 * 2 * 128], FP8,
                          kind="ExternalInput").ap()
    wh_q = nc.dram_tensor("wh_q", [128, 16 * 2 * 2 * 128], FP8,
                          kind="ExternalInput").ap()
    enc_bf = nc.dram_tensor("enc_bf", [128, 16 * U], BF16,
                            kind="ExternalInput").ap()
    # encT fp8 interleaved [p, (kk, c=BL, jj)] x SE
    enct_q = nc.dram_tensor("enct_q", [128, 2 * BL * 2], FP8,
                            kind="ExternalInput").ap()
    # Wa/2 fp8 [p, (mu, kk, jj, m)] x SWA
    wa_q = nc.dram_tensor("wa_q", [128, 4 * 2 * 2 * 128], FP8,
                          kind="ExternalInput").ap()
    h0 = nc.dram_tensor("h0", [B, U], F32, kind="ExternalInput").ap()
    c0 = nc.dram_tensor("c0", [B, U], F32, kind="ExternalInput").ap()
    wfc_q = nc.dram_tensor("wfc_q", [128, 4 * VS * 2], FP8,
                           kind="ExternalInput").ap()
    if lstm_bias:
        wxb_in = nc.dram_tensor("wxb_bf", [1, G4U], BF16,
                                kind="ExternalInput").ap()
    if fc_bias:
        bfc_in = nc.dram_tensor("bfc_bf", [1, VS], BF16,
                                kind="ExternalInput").ap()
    out = nc.dram_tensor("out", [B, T, VS], BF16, kind="ExternalOutput").ap()

    with tile.TileContext(nc) as tc, ExitStack() as perm:
        # ---------------- permanent pools ----------------
        konst = perm.enter_context(tc.tile_pool(name="konst", bufs=1))
        wpool = perm.enter_context(tc.tile_pool(name="wpool", bufs=1))
        hpool = perm.enter_context(tc.tile_pool(name="hpool", bufs=1))
        dram = perm.enter_context(tc.tile_pool(name="dram", bufs=1, space="DRAM"))
        stats = perm.enter_context(tc.tile_pool(name="stats", bufs=1))

        idt = konst.tile([128, 128], BF16)
        make_identity(nc, idt[:])
        negone = konst.tile([128, 1], F32)
        nc.vector.memset(negone[:], -1.0)
        idtf = konst.tile([128, 128], F32)
        make_identity(nc, idtf[:])
        ones_bf = konst.tile([1, 512], BF16)
        nc.vector.memset(ones_bf[:], 1.0)

        wfc_sb = wpool.tile([128, 4 * VS * 2], FP8)
        wfc_k = wfc_sb[:].rearrange("p (j v k) -> p j k v", j=4, k=2)
        wh_sb = wpool.tile([128, 16 * 2 * 2 * 128], FP8)
        wh_k = wh_sb[:].rearrange("p (mi kk j m) -> p mi kk j m",
                                  mi=16, kk=2, j=2)
        if fc_bias:
            bfc_row = wpool.tile([1, VS], BF16)

        # H8: fp8 x16 h (recurrence + scores + FC). col = k*2048+slot*32+b
        H8 = hpool.tile([128, 4 * 64 * B], FP8)
        H8k = H8[:].rearrange("p (k s b) -> p k s b", k=4, s=64)
        H8r = H8[:].rearrange("p (k c) -> p k c", k=4)
        # G_ctx: ctx.T fp8 x16, col = k*2016 + t*32 + b
        Gc = hpool.tile([128, 4 * TB], FP8)
        Gck = Gc[:].rearrange("p (k t b) -> p k t b", k=4, t=T)
        Gcr = Gc[:].rearrange("p (k r) -> p k r", k=4)
        cT = hpool.tile([128, 128], F32)     # (2c).T state, col = k*32+b

        # Zx staged in SBUF: [p, (t, mi, b)] fp8, values x SZX
        zxt_sb = hpool.tile([128, T * 512], FP8)
        zxt_t = zxt_sb[:].rearrange("p (t c) -> p t c", t=T)
        zxt_blk = zxt_sb[:].rearrange("p (t m b) -> p m t b", t=T, m=16)

        # ep fp8 [p, (kk, b, l, jj)] x SEP
        epT_sb = hpool.tile([128, 2 * BL * 2], FP8)
        e8v = epT_sb[:].rearrange("p (kk b l jj) -> p kk jj b l",
                                  kk=2, b=B, jj=2)
        e8w = epT_sb[:].rearrange("p (kk c jj) -> p kk jj c", kk=2, jj=2)
        enc_sb = hpool.tile([128, 16 * U], BF16)    # 2 b per 64-row group

        # per-row ((t,b) grouped [128 x 16]) log-softmax stats.
        lsum_sb = stats.tile([128, 16], F32)   # local sum exp(x - 1)
        sg_sb = stats.tile([128, 16], F32)     # global sum
        logz_sb = stats.tile([128, 16], F32)   # ln(global sum)
        nlz_sb = stats.tile([128, 16], F32)    # -(1 + ln(global sum))
        nc.vector.memset(lsum_sb[:], 1.0)

        # DRAM scratch
        logits_d = dram.tile([16, 128, VS], BF16)    # tanh'd logits
        ccs_in = [dram.tile([128, cb - ca], F32, name=f"cci{i}")
                  for i, (ca, cb) in enumerate(AR_RANGES)]
        ccs_out = [dram.tile([128, cb - ca], F32, name=f"cco{i}")
                   for i, (ca, cb) in enumerate(AR_RANGES)]

        d1w = perm.enter_context(tc.tile_pool(name="d1w", bufs=3))
        d1s = perm.enter_context(tc.tile_pool(name="d1s", bufs=3))
        sst = perm.enter_context(tc.tile_pool(name="sst", bufs=6))

        lg_tiles = {}
        ac_tiles = {}

        # =========== schedulable work units (emitted into R) ===========
        pools = {}

        def fc_q(mi, qi):
            """FC chunk mi, vocab quarter qi: fp8 DoubleRow + tanh + exp."""
            r0 = mi * 128
            rows = min(128, TB - r0)
            if qi == 0:
                lg_tiles[mi] = pools["lgp"].tile([128, VS], BF16, tag="lg",
                                                 name="lg")
                ac_tiles[mi] = []
            lg = lg_tiles[mi]
            base = qi * QTR
            fcp = pools["fps"].tile([128, QTR], F32, tag="fc")
            for j in range(4):
                lhs = Gcr[:, 2 * j: 2 * j + 2, r0:r0 + rows] if j < 2 \
                    else H8r[:, 2 * (j - 2): 2 * (j - 2) + 2,
                             B + r0: B + r0 + rows]
                for off, w in [(0, 512), (512, 488)]:
                    nc.tensor.matmul(
                        fcp[:rows, off:off + w], lhs,
                        wfc_k[:, j, :, base + off: base + off + w],
                        start=(j == 0), stop=(j == 3) and not fc_bias,
                        perf_mode=PM.DoubleRow)
            if fc_bias:
                for off, w in [(0, 512), (512, 488)]:
                    nc.tensor.matmul(
                        fcp[:rows, off:off + w],
                        ones_bf[:1, :rows],
                        bfc_row[:1, base + off: base + off + w],
                        start=False, stop=True,
                        skip_group_check=True)
            qs = slice(base, base + QTR)
            nc.scalar.activation(lg[:rows, qs], fcp[:rows, :], AF.Tanh,
                                 scale=1.0 / SP)
            nc.sync.dma_start(logits_d[mi, :rows, qs], lg[:rows, qs])
            if qi in (1, 3):
                hs = slice((qi // 2) * HALF, (qi // 2) * HALF + HALF)
                sc_ = pools["scr"].tile([128, HALF], BF16, tag="sc")
                acx = sst.tile([128, 1], F32, tag="ac")
                nc.scalar.activation(sc_[:rows, :], lg[:rows, hs], AF.Exp,
                                     bias=negone[:rows, :],
                                     accum_out=acx[:rows, :])
                ac_tiles[mi].append(acx)
            if qi == 3:
                a = ac_tiles[mi]
                nc.vector.tensor_add(lsum_sb[:rows, mi:mi + 1],
                                     a[0][:rows, :], a[1][:rows, :])

        def ar_batch(bi):
            """AllReduce batch bi's sumexp; nlz = -(1 + ln S)."""
            ca, cb = AR_RANGES[bi]
            nc.sync.dma_start(ccs_in[bi][:], lsum_sb[:, ca:cb])
            nc.gpsimd.collective_compute(
                "AllReduce", ALU.add,
                replica_groups=[list(range(n_cores))],
                ins=[ccs_in[bi][:].opt()], outs=[ccs_out[bi][:].opt()])
            nc.gpsimd.dma_start(sg_sb[:, ca:cb], ccs_out[bi][:])
            nc.scalar.activation(logz_sb[:, ca:cb], sg_sb[:, ca:cb], AF.Ln)
            nc.vector.tensor_scalar(nlz_sb[:, ca:cb], logz_sb[:, ca:cb],
                                    -1.0, -1.0, op0=ALU.mult, op1=ALU.add)

        def fin_half(mi, half):
            """out = logits - (1 + lnS), reloading logits from DRAM."""
            r0 = mi * 128
            rows = min(128, TB - r0)
            hs = slice(half * HALF, (half + 1) * HALF)
            ob = pools["fin"].tile([128, HALF], BF16, tag="ob")
            nc.sync.dma_start(ob[:rows, :], logits_d[mi, :rows, hs])
            if mi < 12 or (2 * mi + half) % 2 == 0:
                nc.vector.tensor_scalar(
                    ob[:rows, :], ob[:rows, :],
                    nlz_sb[:rows, mi:mi + 1], None, op0=ALU.add)
            else:
                nc.scalar.activation(ob[:rows, :], ob[:rows, :],
                                     AF.Identity,
                                     bias=nlz_sb[:rows, mi:mi + 1])
            t0 = mi * 4
            eng = nc.gpsimd if (2 * mi + half) % 2 == 0 else nc.sync
            for tl in range(rows // B):
                eng.dma_start(out[:, t0 + tl, hs],
                              ob[tl * B:(tl + 1) * B, :])

        def d1_sub(s0, nt, u):
            """Attention sub-unit: j-pairs 2u, 2u+1 of a step block.

            Block covers h slots s0..s0+nt-1 -> out-t s0-1..s0+nt-2.
            scores (fp8 DR, raw scale) -> softmax -> attn.T -> ctx.T ->
            Gc (fp8 x16).
            """
            for j in (2 * u, 2 * u + 1):
                # both batch-halves at partition base 0 (DR dst rule),
                # separated by 64-column blocks
                scp = pools["pps"].tile([128, 512], F32, tag="zx")
                for hf in range(2):
                    b = 2 * j + hf
                    cs = slice(64 * hf, 64 * hf + 64)
                    for kk in range(2):
                        nc.tensor.matmul(
                            scp[:nt, cs],
                            H8k[:, 2 * kk:2 * kk + 2, s0:s0 + nt, b],
                            e8v[:, kk, :, b, :],
                            start=(kk == 0), stop=(kk == 1),
                            perf_mode=PM.DoubleRow)
                att_f = d1w.tile([128, 128], F32, tag="af")
                attb = d1w.tile([128, 128], BF16, tag="ab")
                for hf in range(2):
                    cs = slice(64 * hf, 64 * hf + 64)
                    nmx = d1s.tile([128, 1], F32, tag="nm")
                    nc.vector.tensor_reduce(nmx[:nt, :], scp[:nt, cs],
                                            axis=AX.X, op=ALU.max,
                                            negate=True)
                    ssum = d1s.tile([128, 1], F32, tag="ss")
                    nc.scalar.activation(att_f[:nt, cs], scp[:nt, cs],
                                         AF.Exp, bias=nmx[:nt, :],
                                         accum_out=ssum[:nt, :])
                    rcp = d1s.tile([128, 1], F32, tag="rc")
                    nc.vector.reciprocal(rcp[:nt, :], ssum[:nt, :])
                    nc.vector.tensor_scalar_mul(attb[:nt, cs],
                                                att_f[:nt, cs],
                                                rcp[:nt, :])
                atT = d1w.tile([128, 16], BF16, tag="atT")
                for hf in range(2):
                    po = 64 * hf
                    cs = slice(64 * hf, 64 * hf + 64)
                    tpp = pools["tpsA"].tile([128, 128], BF16, tag="tpb")
                    nc.tensor.transpose(tpp[po:po + L, :nt],
                                        attb[:nt, cs],
                                        idt[:nt, :nt])
                    nc.vector.tensor_copy(atT[po:po + L, :nt],
                                          tpp[po:po + L, :nt])
                for hf in range(2):
                    b = 2 * j + hf
                    po = 64 * hf
                    ctp = pools["pps"].tile([128, 512], F32, tag="zx")
                    for mu in range(4):
                        nc.tensor.matmul(
                            ctp[:, mu * nt:(mu + 1) * nt],
                            enc_sb[po:po + L,
                                   j * U + mu * 128: j * U + (mu + 1) * 128],
                            atT[po:po + L, :nt],
                            start=True, stop=True)
                    nc.vector.tensor_scalar_mul(
                        Gck[:, :, s0 - 1: s0 - 1 + nt, b],
                        ctp[:, :4 * nt].rearrange("p (k tt) -> p k tt", k=4),
                        SA)

        # ================================================================
        with ExitStack() as pscope:
            psb = pscope.enter_context(tc.tile_pool(name="p_sbuf", bufs=2))
            pps = pscope.enter_context(
                tc.tile_pool(name="p_psum", bufs=2, space="PSUM"))
            pools["pps"] = pps
            rzp = pscope.enter_context(
                tc.tile_pool(name="r_zps", bufs=1, space="PSUM"))
            pools["fps"] = pscope.enter_context(
                tc.tile_pool(name="fc_psum", bufs=2, space="PSUM"))
            pools["tpsA"] = pscope.enter_context(
                tc.tile_pool(name="tpa_ps", bufs=1, space="PSUM"))
            rga = pscope.enter_context(tc.tile_pool(name="r_gate", bufs=2))

            # pools released mid-R to free SBUF
            zscope = ExitStack()
            zwp = zscope.enter_context(tc.tile_pool(name="zwp", bufs=1))
            # x.T fp8 interleaved: col = row*2 + j  (j = E-chunk), x SX
            xt_sb = zwp.tile([128, 2 * TB], FP8)
            xt_j = xt_sb[:].rearrange("p (r j) -> p j r", j=2)
            wx_sb = zwp.tile([128, 16 * 2 * 128], FP8)
            wx_k = wx_sb[:].rearrange("p (mi j m) -> p mi j m", mi=16, j=2)
            if lstm_bias:
                wxb_sb = zwp.tile([1, G4U], BF16)
            escope = ExitStack()
            ewp = escope.enter_context(tc.tile_pool(name="ewp", bufs=1))
            enct_sb = ewp.tile([128, 2 * BL * 2], FP8)
            e8in = enct_sb[:].rearrange("p (kk c jj) -> p kk jj c",
                                        kk=2, jj=2)
            wa_sb = ewp.tile([128, 4 * 2 * 2 * 128], FP8)
            wa_k = wa_sb[:].rearrange("p (mu kk jj m) -> p mu kk jj m",
                                      mu=4, kk=2, jj=2)

            def emit_zx_unit(nb, mi):
                """One (t-block, m-chunk) unit of Zx.T = W_x.T@X.T (+b)."""
                t0 = nb * 16
                tn = min(16, T - t0)
                ncols = tn * B
                zps = pps.tile([128, 512], F32, tag="zx")
                nc.tensor.matmul(
                    zps[:, :ncols],
                    wx_k[:, mi],
                    xt_j[:, :, t0 * B: t0 * B + ncols],
                    start=True, stop=not lstm_bias,
                    perf_mode=PM.DoubleRow)
                if lstm_bias:
                    nc.tensor.matmul(zps[:, :ncols],
                                     wxb_sb[:1, mi * 128:(mi + 1) * 128],
                                     ones_bf[:1, :ncols],
                                     start=False, stop=True,
                                     skip_group_check=True)
                nc.vector.tensor_scalar_mul(
                    zxt_blk[:, mi, t0:t0 + tn, :],
                    zps[:, :ncols].rearrange("p (t b) -> p t b", b=B),
                    SZX / (SWX * SX))

            def emit_ep_unit(un):
                """ep8 = (enc @ Wa/2) x SEP, unit (mu, nb), fp8 DR."""
                mu, nb = un // 4, un % 4
                kk, jj = mu // 2, mu % 2
                eps_ = pps.tile([128, 512], F32, tag="zx")
                for k2 in range(2):
                    nc.tensor.matmul(
                        eps_[:, :],
                        wa_k[:, mu, k2],
                        e8in[:, k2, :, nb * 512:(nb + 1) * 512],
                        start=(k2 == 0), stop=(k2 == 1),
                        perf_mode=PM.DoubleRow)
                nc.vector.tensor_scalar_mul(
                    e8w[:, kk, jj, nb * 512:(nb + 1) * 512],
                    eps_[:], SEP / (SWA * SE))

            # ============ phase P: minimal preamble ============
            def emit_gather(i):
                r0 = i * 128
                rows = min(128, TB - r0)
                ix = psb.tile([128, 1], I32, tag="ix")
                nc.sync.dma_start(ix[:rows, :], tidx[r0:r0 + rows, :])
                xg = psb.tile([128, E], BF16, tag="xg")
                nc.gpsimd.indirect_dma_start(
                    out=xg[:rows, :], out_offset=None,
                    in_=emb_bf[:],
                    in_offset=bass.IndirectOffsetOnAxis(
                        ap=ix[:rows, :1], axis=0),
                )
                for cc in range(2):
                    tp = pools["tpsA"].tile([128, 128], BF16, tag="tpb")
                    nc.tensor.transpose(
                        tp[:, :rows],
                        xg[:rows, cc * 128:(cc + 1) * 128],
                        idt[:rows, :rows])
                    nc.vector.tensor_scalar_mul(
                        xt_j[:, cc, r0: r0 + rows],
                        tp[:, :rows], SX)

            # big packed weight loads first (spread across DGE queues)
            nc.sync.dma_start(wx_sb[:], wx_q[:])
            nc.scalar.dma_start(wh_sb[:], wh_q[:])
            if lstm_bias:
                nc.sync.dma_start(wxb_sb[:], wxb_in[:])

            for i in range(4):
                emit_gather(i)

            # h0/c0 init: H8 = SA*h, cT = 2c
            hc_sb = psb.tile([B, U], F32, tag="hc")
            nc.sync.dma_start(hc_sb[:, :], h0[:, :])
            cc_sb = psb.tile([B, U], F32, tag="hc2")
            nc.sync.dma_start(cc_sb[:, :], c0[:, :])
            for k in range(4):
                tp = pps.tile([128, 512], F32, tag="zx")
                nc.tensor.transpose(
                    tp[:, :B], hc_sb[:B, k * 128:(k + 1) * 128],
                    idtf[:B, :B])
                nc.vector.tensor_scalar_mul(H8k[:, k, 0, :],
                                            tp[:, :B], SA)
                tp2 = pps.tile([128, 512], F32, tag="zx")
                nc.tensor.transpose(
                    tp2[:, :B], cc_sb[:B, k * 128:(k + 1) * 128],
                    idtf[:B, :B])
                nc.vector.tensor_scalar_mul(
                    cT[:, k * B:(k + 1) * B], tp2[:, :B], 2.0)

            for mi in range(16):
                emit_zx_unit(0, mi)

            # remaining loads (all overlap the early recurrence)
            nc.scalar.dma_start(wfc_sb[:], wfc_q[:])
            if fc_bias:
                nc.sync.dma_start(bfc_row[:, :], bfc_in[:, :])
            nc.scalar.dma_start(enct_sb[:], enct_q[:])
            nc.sync.dma_start(wa_sb[:], wa_q[:])
            nc.sync.dma_start(enc_sb[:], enc_bf[:])

            # ============ phase R: the master pipeline ============
            if True:
                # ---- interleave schedule: step -> work units ----
                sched = defaultdict(list)
                for t in range(0, 6):       # gathers 4..15
                    sched[t] += [lambda i=4 + 2 * t_ + k_: emit_gather(i)
                                 for t_, k_ in [(t, 0), (t, 1)]]
                for t in range(2, 10):      # Zx block 1 + ep 0..7
                    g = t - 2
                    sched[t] += [lambda m=2 * g: emit_zx_unit(1, m),
                                 lambda m=2 * g + 1: emit_zx_unit(1, m),
                                 lambda un=g: emit_ep_unit(un)]
                for t in range(10, 18):     # Zx block 2 + ep 8..15
                    g = t - 10
                    sched[t] += [lambda m=2 * g: emit_zx_unit(2, m),
                                 lambda m=2 * g + 1: emit_zx_unit(2, m),
                                 lambda un=8 + g: emit_ep_unit(un)]
                sched[18] += [escope.close]
                for t in range(16, 20):     # D1 block A (t 0..15)
                    g = t - 16
                    sched[t] += [lambda u=2 * g: d1_sub(1, 16, u),
                                 lambda u=2 * g + 1: d1_sub(1, 16, u)]

                def open_fc_pools():
                    pools["lgp"] = zscope.enter_context(
                        tc.tile_pool(name="lgp", bufs=2))
                    pools["scr"] = zscope.enter_context(
                        tc.tile_pool(name="scr", bufs=2))
                    pools["fin"] = zscope.enter_context(
                        tc.tile_pool(name="fin", bufs=3))
                sched[19] += [open_fc_pools]
                for t in range(20, 28):     # FC chunks 0..3 (16 q)
                    g = t - 20
                    sched[t] += [lambda mi=g // 2, qi=2 * (g % 2):
                                 fc_q(mi, qi),
                                 lambda mi=g // 2, qi=2 * (g % 2) + 1:
                                 fc_q(mi, qi)]
                for t in range(28, 36):     # Zx block 3
                    g = t - 28
                    sched[t] += [lambda m=2 * g: emit_zx_unit(3, m),
                                 lambda m=2 * g + 1: emit_zx_unit(3, m)]
                for t in range(32, 36):     # D1 block B (t 16..31)
                    g = t - 32
                    sched[t] += [lambda u=2 * g: d1_sub(17, 16, u),
                                 lambda u=2 * g + 1: d1_sub(17, 16, u)]
                for t in range(36, 44):     # FC chunks 4..7
                    g = t - 36
                    sched[t] += [lambda mi=4 + g // 2, qi=2 * (g % 2):
                                 fc_q(mi, qi),
                                 lambda mi=4 + g // 2, qi=2 * (g % 2) + 1:
                                 fc_q(mi, qi)]
                sched[44] += [lambda: ar_batch(0)]
                for t in range(48, 52):     # D1 block C (t 32..47)
                    g = t - 48
                    sched[t] += [lambda u=2 * g: d1_sub(33, 16, u),
                                 lambda u=2 * g + 1: d1_sub(33, 16, u)]
                for t in range(47, 62):     # fins chunks 0..7 (batch 0)
                    g = t - 47
                    if g < 16:
                        sched[t] += [lambda mi=g // 2, hf=g % 2:
                                     fin_half(mi, hf)]
                sched[62] += [lambda: fin_half(7, 1)]
                for t in range(52, 56):     # FC chunks 8..9
                    g = t - 52
                    sched[t] += [lambda mi=8 + g // 2, qi=2 * (g % 2):
                                 fc_q(mi, qi),
                                 lambda mi=8 + g // 2, qi=2 * (g % 2) + 1:
                                 fc_q(mi, qi)]
                for t in range(56, 60):     # D1 block D (t 48..55)
                    g = t - 56
                    sched[t] += [lambda u=2 * g: d1_sub(49, 8, u),
                                 lambda u=2 * g + 1: d1_sub(49, 8, u)]
                for t in range(60, 63):     # FC chunk 10 + 11.0
                    g = t - 60
                    sched[t] += [lambda mi=10 + g // 2, qi=2 * (g % 2):
                                 fc_q(mi, qi),
                                 lambda mi=10 + g // 2, qi=2 * (g % 2) + 1:
                                 fc_q(mi, qi)]

                c_prev = cT
                for t in range(T):
                    for _ in range(FILLER):
                        nc.tensor.ldweights(idt[:])
                    zps = rzp.tile([128, 512], F32, tag="zt")
                    # gate order [g, i, f, o] (host-permuted, g cols x2);
                    # sigmoid(z) = (1+tanh(z/2))/2, folded.
                    gate = {}
                    for gi in range(4):
                        for m2 in range(4):
                            mi = gi * 4 + m2
                            for kk in range(2):
                                nc.tensor.matmul(
                                    zps[:, mi * B:(mi + 1) * B],
                                    wh_k[:, mi, kk],
                                    H8k[:, 2 * kk:2 * kk + 2, t, :],
                                    start=(kk == 0), stop=(kk == 1),
                                    perf_mode=PM.DoubleRow)
                        sl = slice(gi * 128, (gi + 1) * 128)
                        zq = rga.tile([128, 128], F32, tag=f"q{gi}",
                                      name=f"q{gi}")
                        nc.vector.scalar_tensor_tensor(
                            zq[:], zxt_t[:, t, sl], SZ / SZX,
                            zps[:, sl], op0=ALU.mult, op1=ALU.add)
                        gt = rga.tile([128, 128], F32, tag=f"g{gi}",
                                      name=f"g{gi}")
                        nc.scalar.activation(gt[:], zq[:], AF.Tanh,
                                             scale=0.5 / SZ)
                        gate[gi] = gt
                        if gi == 1:
                            ig = rga.tile([128, 128], F32, tag="ig")
                            nc.vector.scalar_tensor_tensor(
                                ig[:], gate[1][:], 1.0, gate[0][:],
                                op0=ALU.add, op1=ALU.mult)
                        elif gi == 2:
                            fc_ = rga.tile([128, 128], F32, tag="fc")
                            nc.vector.scalar_tensor_tensor(
                                fc_[:], gate[2][:], 1.0, c_prev[:],
                                op0=ALU.add, op1=ALU.mult)
                            c_new = rga.tile([128, 128], F32, tag="cn")
                            nc.vector.scalar_tensor_tensor(
                                c_new[:], fc_[:], 0.5, ig[:],
                                op0=ALU.mult, op1=ALU.add)
                            tc_ = rga.tile([128, 128], F32, tag="tc")
                            nc.scalar.activation(tc_[:], c_new[:],
                                                 AF.Tanh, scale=0.5)
                    # H8(t+1) = SA*h = ((SA/2)(1+to)) * tanh(c)
                    g3p = rga.tile([128, 128], F32, tag="g3p")
                    nc.vector.tensor_scalar(g3p[:], gate[3][:],
                                            SA / 2.0, SA / 2.0,
                                            op0=ALU.mult, op1=ALU.add)
                    nc.vector.tensor_tensor(
                        H8k[:, :, t + 1, :],
                        g3p[:].rearrange("p (k b) -> p k b", k=4),
                        tc_[:].rearrange("p (k b) -> p k b", k=4),
                        op=ALU.mult)
                    c_prev = c_new
                    for unit in sched[t]:
                        unit()

                # =================== tail ===================
                fc_q(11, 2)
                fc_q(11, 3)
                for g in range(4):          # FC chunks 12..13
                    fc_q(12 + g // 2, 2 * (g % 2))
                    fc_q(12 + g // 2, 2 * (g % 2) + 1)
                ar_batch(1)                 # chunks 8..13
                for u in range(8):          # D1 block E (t 56..62)
                    d1_sub(57, 7, u)
                    if u % 2 == 1:
                        fin_half(8 + u // 2, 0)
                        fin_half(8 + u // 2, 1)
                for g in range(4):          # FC chunks 14..15
                    fc_q(14 + g // 2, 2 * (g % 2))
                    fc_q(14 + g // 2, 2 * (g % 2) + 1)
                    if g >= 2:
                        fin_half(10 + g, 0)
                        fin_half(10 + g, 1)
                ar_batch(2)                 # chunks 14..15
                for mi in range(14, 16):
                    for hf in range(2):
                        fin_half(mi, hf)
                zscope.close()

    nc.compile()
    return nc


def _bf(x):
    return np.ascontiguousarray(
        np.asarray(x, np.float32).astype(ml_dtypes.bfloat16))


def _q8(x, scale):
    y = np.asarray(x, np.float32) * scale
    y = np.clip(y, -240.0, 240.0)
    return np.ascontiguousarray(y.astype(ml_dtypes.float8_e4m3))


def prep_inputs(target, encoder_outputs, enc_h0, enc_c0, emb, W_x, W_h,
                b_lstm, Wa, W_fc, b_fc, n_cores=NC):
    """Host-side layout prep + per-core sharding."""
    tgt = np.asarray(target).astype(np.int32)
    tidx = np.ascontiguousarray(tgt[:, :T].T.reshape(TB, 1))  # t-major rows
    enc = np.asarray(encoder_outputs, np.float32)
    # permute gate columns [i,f,g,o] -> [g,i,f,o]
    gperm = np.r_[2 * U:3 * U, 0:U, U:2 * U, 3 * U:4 * U]
    # double the g-gate block so every gate uses the tanh(z/2) scale
    gscale = np.ones(G4U, np.float32)
    gscale[0:U] = 2.0
    Wxp = np.asarray(W_x, np.float32)[:, gperm] * gscale
    # wx fp8 packed [128, (mi, j, m)]
    wx_pack = Wxp.reshape(2, 128, 16, 128).transpose(1, 2, 0, 3) \
        .reshape(128, 16 * 2 * 128)
    # W_h fp8 packed [128, (mi, kk, j, m)]
    Whp = np.asarray(W_h, np.float32)[:, gperm] * gscale  # [512, 2048]
    wh = Whp.reshape(4, 128, 16, 128)                    # [kt, p, mi, m]
    wh = wh.reshape(2, 2, 128, 16, 128)                  # [kk, j, p, mi, m]
    wh = wh.transpose(2, 3, 0, 1, 4).reshape(128, 16 * 2 * 2 * 128)
    # enc packed: enc_sb[0:64, j*U:] = enc[2j], [64:128] = enc[2j+1]
    enc_pack = np.empty((128, 16 * U), np.float32)
    for j in range(16):
        enc_pack[0:64, j * U:(j + 1) * U] = enc[2 * j]
        enc_pack[64:128, j * U:(j + 1) * U] = enc[2 * j + 1]
    # encT fp8 interleaved [p, (kk, c, jj)]: elem = enc.T[(2kk+jj)*128+p, c]
    enct = enc.transpose(2, 0, 1).reshape(U, BL)         # [U, (b,l)]
    e4 = enct.reshape(2, 2, 128, BL)                     # [kk, jj, p, c]
    enct_pack = e4.transpose(2, 0, 3, 1).reshape(128, 2 * BL * 2)
    # Wa/2 fp8 [p, (mu, kk, jj, m)]: row = (2kk+jj)*128+p, col = mu*128+m
    wa = np.asarray(Wa, np.float32) * 0.5
    wa4 = wa.reshape(2, 2, 128, 4, 128)                  # [kk, jj, p, mu, m]
    wa_pack = wa4.transpose(2, 3, 0, 1, 4).reshape(128, 4 * 2 * 2 * 128)
    b_lstm = np.asarray(b_lstm, np.float32)
    lstm_bias = bool(np.any(b_lstm))
    common = {
        "tidx": tidx,
        "emb_bf": _bf(emb),
        "wx_q": _q8(wx_pack, SWX),
        "wh_q": _q8(wh, SWH),
        "enc_bf": _bf(enc_pack),
        "enct_q": _q8(enct_pack, SE),
        "wa_q": _q8(wa_pack, SWA),
        "h0": np.ascontiguousarray(np.asarray(enc_h0, np.float32)),
        "c0": np.ascontiguousarray(np.asarray(enc_c0, np.float32)),
    }
    if lstm_bias:
        # lands in the zx PSUM, which carries scale SWX*SX
        common["wxb_bf"] = _bf(b_lstm[gperm] * gscale * SWX * SX)
        common["wxb_bf"] = common["wxb_bf"].reshape(1, G4U)
    wfc = np.asarray(W_fc, np.float32)
    bfc = np.asarray(b_fc, np.float32)
    fc_bias = bool(np.any(bfc))
    in_maps = []
    for c in range(n_cores):
        m = dict(common)
        wshard = wfc[:, c * VS:(c + 1) * VS]             # [1024, VS]
        wsh = wshard.reshape(4, 2, 128, VS)              # [j, kk, p, v]
        wsh = wsh.transpose(2, 0, 3, 1).reshape(128, 4 * VS * 2)
        m["wfc_q"] = _q8(wsh, SW)
        if fc_bias:
            m["bfc_bf"] = _bf(bfc[c * VS:(c + 1) * VS].reshape(1, VS) * SP)
        in_maps.append(m)
    return in_maps, fc_bias, lstm_bias


def kernel(**inputs):
    in_maps, fc_bias, lstm_bias = prep_inputs(**inputs, n_cores=NC)
    key = ("nc", fc_bias, lstm_bias)
    if key not in _CACHE:
        _CACHE[key] = build(NC, fc_bias=fc_bias, lstm_bias=lstm_bias)
        _CACHE["nc"] = _CACHE[key]
    nc = _CACHE[key]
    res = run_bass_kernel_spmd(nc, in_maps, list(range(NC)))
    shards = [np.asarray(res.results[c]["out"]).astype(np.float32)
              for c in range(NC)]
    return np.concatenate(shards, axis=-1)
